# revision 1
# baseline (speedup 1.0000x reference)
"""Trainium2 Bass kernel for the hyperbolic (Poincare-ball) GRU cell.

Data-parallel over batch across 8 NeuronCores, no collectives.

Numerics (validated in numpy sim to 8.4e-4 vs the fp32 jax reference):
  - zero_log(x) @ W = s_x * (x @ W): log-map scaling commutes with the GEMM.
  - mobius_add of scaled vectors reduces to per-row scalar pairs (ua, ub), and
    |ua*a + ub*b|^2 = ua^2|a|^2 + 2 ua ub <a,b> + ub^2|b|^2 -- norms of every
    mobius combination are DERIVED from scalars instead of re-reduced.
  - e = z*d with d = ua*hx + ub*q is rewritten e = ua*(z*hx) + ub*(z*q); d is
    never materialized and |e|^2 / <hx,e> are scalar-derived.
  - GEMMs 1-4 (r/z sigmoid branches) in fp8 e4m3 DoubleRow (2x PE rate); the
    sigmoid slope damps the quantization noise (sim-verified). GEMMs 5-6 (the
    tanh path) in fp16.
  - Biases enter via a rank-1 matmul into the same PSUM group (lhsT = per-row
    recip scale, rhs = bias row), so PSUM->SBUF copies are pure ACT scales.
  - Intermediates fp16 (2x DVE rates); tanh(sqrt u)/sqrt u and
    artanh(sqrt u)/sqrt u are deg-9 polynomials on DVE; ACT only runs
    Copy/Square/Sigmoid/Tanh -> one activation-table set.
  - All weights/activations are pre-swizzled on the host into the exact slab
    layouts ([NJ, P, KC, JB] / [P, KC, BL]) so every DMA reads fully
    contiguous per-partition lines (DMA_DIRECT2D transfers execute serially
    on the Sync engine; line size sets the effective bandwidth).
"""

import threading
from contextlib import ExitStack

import ml_dtypes
import numpy as np

import concourse.bacc as bacc
import concourse.mybir as mybir
import concourse.tile as tile
from concourse.bass_utils import run_bass_kernel_spmd
from concourse.masks import make_identity

F32 = mybir.dt.float32
F16 = mybir.dt.float16
BF16 = mybir.dt.bfloat16
F8 = mybir.dt.float8e4
AF = mybir.ActivationFunctionType
OP = mybir.AluOpType
PM = mybir.MatmulPerfMode

N_CORES = 8
B, D = 4096, 2048
BL = B // N_CORES          # 512 rows per core
P = 128                    # partitions
NB = BL // P               # 4 batch tiles per core
KC = D // P                # 16 contraction chunks
JB = 512                   # GEMM j-block width
NJ = D // JB               # 4 j-blocks

SA = 2048.0                # fp8 activation scale
SW = 1024.0                # fp8 weight scale
RS8 = SA * SW              # 2^21, folded out of fp8 GEMM copies
SB8 = 2048.0               # fp8 bias-row scale

EPS = 1e-5

# t(u) = tanh(sqrt u)/sqrt u on [0, 0.95]; g(u) = artanh(sqrt u)/sqrt u on
# [0.10, 0.80]; max errs 2e-11 / 6.9e-6 (Chebyshev deg 9).
T_COEF = [1.0, -0.333333329, 0.133333183, -0.05396619044, 0.02185495027,
          -0.008803496923, 0.003439097315, -0.001203079678, 0.0003169332143,
          -4.391237846e-05]
G_COEF = [0.9986986426, 0.3781016454, -0.4412363167, 5.173515875, -23.8123998,
          72.00036756, -137.3859594, 162.2957814, -108.0106185, 31.34506744]


def _build():
    nc = bacc.Bacc(None, target_bir_lowering=False, debug=False)

    hx16_d = nc.dram_tensor("hx16", [P, NB, D], F16, kind="ExternalInput")
    x16_d = nc.dram_tensor("x16", [P, NB, D], F16, kind="ExternalInput")
    xT8_d = nc.dram_tensor("xT8", [P, KC, BL], F8, kind="ExternalInput")
    hxT8_d = nc.dram_tensor("hxT8", [P, KC, BL], F8, kind="ExternalInput")
    xT16_d = nc.dram_tensor("xT16", [P, KC, BL], F16, kind="ExternalInput")
    w8_d = {
        name: nc.dram_tensor(name, [NJ, P, KC, JB], F8, kind="ExternalInput")
        for name in ["wr8", "wz8", "ur8", "uz8"]
    }
    w16_d = {
        name: nc.dram_tensor(name, [NJ, P, KC, JB], F16, kind="ExternalInput")
        for name in ["uw16", "ww16"]
    }
    brz8_d = nc.dram_tensor("brz8", [1, 2 * D], F8, kind="ExternalInput")
    bw16_d = nc.dram_tensor("bw16", [1, D], F16, kind="ExternalInput")
    out_d = nc.dram_tensor("out", [BL, D], F32, kind="ExternalOutput")

    with ExitStack() as ctx:
        tc = ctx.enter_context(tile.TileContext(nc))
        dve, sca, pe = nc.vector, nc.scalar, nc.tensor

        # ---------------- persistent pools ----------------
        perm = ctx.enter_context(tc.tile_pool(name="perm", bufs=1))
        scal = ctx.enter_context(tc.tile_pool(name="scal", bufs=1))
        slot = ctx.enter_context(tc.tile_pool(name="slot", bufs=1))
        hxp = ctx.enter_context(tc.tile_pool(name="hxp", bufs=1))
        ptp = ctx.enter_context(tc.tile_pool(name="ptp", bufs=1))
        outp = ctx.enter_context(tc.tile_pool(name="outp", bufs=1))
        junkp = ctx.enter_context(tc.tile_pool(name="junkp", bufs=2))
        ww01p = ctx.enter_context(tc.tile_pool(name="ww01p", bufs=1))
        pmm = ctx.enter_context(tc.tile_pool(name="pmm", bufs=7, space="PSUM"))
        ptr = ctx.enter_context(tc.tile_pool(name="ptr", bufs=1, space="PSUM"))

        ident16 = perm.tile([P, P], F16, tag="id16", name="id16")
        make_identity(nc, ident16)

        def stile(w=NB, name="s", pt=False):
            # pt=True -> unique persistent tag (long-lived); else recycled ring
            if pt:
                return scal.tile([P, w], F32, tag=f"P_{name}", bufs=1, name=name)
            return scal.tile([P, w], F32, tag=f"scal{w}",
                             bufs=(64 if w == NB else 16), name=name)

        def junk():
            return junkp.tile([P, D], F16, tag="junk", bufs=2, name="junk")

        # big fp16 working slots: 4 groups, each one [P, NB, D] tile
        def slot_group(nm):
            t = slot.tile([P, NB, D], F16, tag=nm, name=nm)
            return t, [t[:, bt, :] for bt in range(NB)]

        A_big, A = slot_group("A")
        B_big, Bs = slot_group("B")
        C_big, C = slot_group("C")
        D_big, Ds = slot_group("D")
        hx_t = hxp.tile([P, NB, D], F16, tag="hx", name="hx")
        hx = [hx_t[:, bt, :] for bt in range(NB)]
        pT = ptp.tile([P, KC, BL], F16, tag="pT", name="pT")
        outt = [outp.tile([P, D], F32, tag=f"o{i}", name=f"o{i}")
                for i in range(2)]


        # ---------------- tiny-op helpers ----------------
        def sq_accum_dve(t, acc_col):
            dve.scalar_tensor_tensor(junk(), t, 1.0, t, OP.mult, OP.mult,
                                     accum_out=acc_col)

        def sq_accum_act(t, acc_col):
            sca.activation(junk(), t, AF.Square, accum_out=acc_col)

        def dot_accum(a, b, acc_col):
            dve.scalar_tensor_tensor(junk(), a, 1.0, b, OP.mult, OP.mult,
                                     accum_out=acc_col)

        def dot_accum_split(a, b, acc_col, scratch):
            dve.tensor_tensor(scratch, a, b, OP.mult)
            sca.activation(junk(), scratch, AF.Copy, accum_out=acc_col)

        def poly(dst, u, coefs):
            dve.tensor_scalar(dst, u, float(coefs[-1]), float(coefs[-2]),
                              OP.mult, OP.add)
            for ck in reversed(coefs[:-2]):
                dve.tensor_tensor(dst, dst, u, OP.mult)
                dve.tensor_scalar(dst, dst, float(ck), None, OP.add)

        def t_poly(u, w=NB, pt=False, name="tp"):
            uc = stile(w, "uc")
            dve.tensor_scalar(uc, u, 0.95, None, OP.min)
            o = stile(w, name, pt=pt)
            poly(o, uc, T_COEF)
            return o

        def g_poly(u, w=NB, pt=False, name="gp"):
            uc = stile(w, "ug")
            dve.tensor_scalar(uc, u, 0.10, 0.80, OP.max, OP.min)
            o = stile(w, name, pt=pt)
            poly(o, uc, G_COEF)
            return o

        def mobius(al_a, x2, al_b, y2, dab, n2a, n2b, neg_a=False, pfx="m"):
            """Returns ua, ub, n2m = |ua*a + ub*b|^2 (width from dab)."""
            w = dab.shape[-1]
            xy = stile(w, name="xy")
            dve.tensor_tensor(xy, al_a, al_b, OP.mult)
            dve.tensor_tensor(xy, xy, dab, OP.mult)
            if neg_a:
                dve.tensor_scalar(xy, xy, -1.0, None, OP.mult)
            txy1 = stile(w, name="txy1")
            dve.tensor_scalar(txy1, xy, 2.0, 1.0, OP.mult, OP.add)
            numa = stile(w, name="numa")
            dve.tensor_tensor(numa, txy1, y2, OP.add)
            den = stile(w, name="den")
            dve.tensor_tensor(den, x2, y2, OP.mult)
            dve.tensor_tensor(den, den, txy1, OP.add)
            dve.tensor_scalar(den, den, float(EPS), None, OP.max)
            rden = stile(w, name="rden")
            dve.reciprocal(rden, den)
            ua = stile(w, name=f"{pfx}_ua", pt=True)
            dve.tensor_tensor(ua, numa, al_a, OP.mult)
            dve.tensor_tensor(ua, ua, rden, OP.mult)
            if neg_a:
                dve.tensor_scalar(ua, ua, -1.0, None, OP.mult)
            ub = stile(w, name=f"{pfx}_ub", pt=True)
            dve.tensor_scalar(ub, x2, -1.0, 1.0, OP.mult, OP.add)
            dve.tensor_tensor(ub, ub, al_b, OP.mult)
            dve.tensor_tensor(ub, ub, rden, OP.mult)
            t1 = stile(w, name="t1")
            dve.tensor_tensor(t1, ua, ua, OP.mult)
            dve.tensor_tensor(t1, t1, n2a, OP.mult)
            t2 = stile(w, name="t2")
            dve.tensor_tensor(t2, ub, ub, OP.mult)
            dve.tensor_tensor(t2, t2, n2b, OP.mult)
            t3 = stile(w, name="t3")
            dve.tensor_tensor(t3, ua, ub, OP.mult)
            dve.tensor_tensor(t3, t3, dab, OP.mult)
            n2m = stile(w, name=f"{pfx}_n2m", pt=True)
            dve.scalar_tensor_tensor(n2m, t3, 2.0, t1, OP.mult, OP.add)
            dve.tensor_tensor(n2m, n2m, t2, OP.add)
            return ua, ub, n2m

        def branch(n2ab, dab, pfx):
            """n2ab: [P, 2w] (first half |a|^2, second |b|^2) -> ua, ub, beta."""
            w = dab.shape[-1]
            al = t_poly(n2ab, w=2 * w)
            x2y2 = stile(2 * w, "x2y2")
            dve.tensor_tensor(x2y2, al, al, OP.mult)
            dve.tensor_tensor(x2y2, x2y2, n2ab, OP.mult)
            ua, ub, n2m = mobius(al[:, 0:w], x2y2[:, 0:w],
                                 al[:, w:], x2y2[:, w:],
                                 dab, n2ab[:, 0:w], n2ab[:, w:], pfx=pfx)
            return ua, ub, g_poly(n2m, w=w, pt=True, name=f"{pfx}_beta")

        def combine(dst, a, ua_col, b, ub_col):
            """dst = ua*a + ub*b; b is scaled in place."""
            dve.tensor_scalar(b, b, ub_col, None, OP.mult)
            dve.scalar_tensor_tensor(dst, a, ua_col, b, OP.mult, OP.add)

        n2xh = stile(2 * NB, "n2xh", pt=True)   # cols 0..3 |x|^2, 4..7 |hx|^2

        def make_sx():
            s_x = g_poly(n2xh[:, 0:NB], pt=True, name="s_x")
            sc8_x = stile(name="sc8_x", pt=True)
            dve.tensor_scalar(sc8_x, s_x, 1.0 / RS8, None, OP.mult)
            # recip rows for the rank-1 bias matmuls: DRAM-bounce the per-row
            # scalars into partition 0 so lhsT slices sit at base partition 0.
            rt_src = scal.tile([P, 2 * NB], F32, tag="rt_src", name="rt_src")
            rsx = stile(name="rsx", pt=True)
            dve.reciprocal(rsx, s_x)
            dve.tensor_scalar(rt_src[:, 0:NB], rsx, RS8 / SB8, None, OP.mult)
            dve.tensor_copy(out=rt_src[:, NB:2 * NB], in_=rsx)
            rt8_src = scal.tile([P, NB], BF16, tag="rt8_src", bufs=1, name="rt8")
            dve.tensor_copy(out=rt8_src, in_=rt_src[:, 0:NB])
            rt5_src = scal.tile([P, NB], F16, tag="rt5_src", bufs=1, name="rt5")
            dve.tensor_copy(out=rt5_src, in_=rt_src[:, NB:2 * NB])
            with tc.tile_pool(name="dscr", bufs=1, space="DRAM") as dscr:
                dt8 = dscr.tile([NB, P], BF16, tag="dt8", name="dt8")
                dt5 = dscr.tile([NB, P], F16, tag="dt5", name="dt5")
                nc.gpsimd.dma_start(out=dt8[:, :].rearrange("c p -> p c"),
                                    in_=rt8_src)
                nc.gpsimd.dma_start(out=dt5[:, :].rearrange("c p -> p c"),
                                    in_=rt5_src)
                r8 = perm.tile([1, NB * P], BF16, tag="recT8", name="recT8")
                r5 = perm.tile([1, NB * P], F16, tag="recT5", name="recT5")
                nc.gpsimd.dma_start(out=r8,
                                    in_=dt8[:, :].rearrange("c p -> (c p)"))
                nc.gpsimd.dma_start(out=r5,
                                    in_=dt5[:, :].rearrange("c p -> (c p)"))
            return s_x, sc8_x, r8, r5

        # ---------------- GEMM machinery ----------------
        def load_aT(pool, src, dtype, nm):
            t = pool.tile([P, KC, BL], dtype, tag=f"aT_{nm}", name=nm)
            nc.sync.dma_start(out=t, in_=src[:, :, :])
            return t

        def gemm_fp8(wpool, wt_dram, actT, scale, v_dst,
                     brow=None, rec=None):
            for js in range(NJ):
                slab = wpool.tile([P, KC, JB], F8, tag="w8", bufs=3, name="w8")
                nc.sync.dma_start(out=slab, in_=wt_dram[js, :, :, :])
                for bt in range(NB):
                    ps = pmm.tile([P, JB], F32, tag="mm", name="mm")
                    first = True
                    if brow is not None:
                        pe.matmul(ps, rec[0:1, bt * P:(bt + 1) * P],
                                  brow[0:1, js * JB:(js + 1) * JB],
                                  start=True, stop=False)
                        first = False
                    for c in range(KC // 2):
                        pe.matmul(
                            ps,
                            actT[:, 2 * c:2 * c + 2, bt * P:(bt + 1) * P],
                            slab[:, 2 * c:2 * c + 2, :],
                            start=(first and c == 0),
                            stop=(c == KC // 2 - 1),
                            perf_mode=PM.DoubleRow,
                        )
                    sca.activation(
                        out=v_dst[bt][:, js * JB:(js + 1) * JB], in_=ps,
                        func=AF.Copy, scale=scale[:, bt:bt + 1],
                    )

        def gemm_f16(wpool, wt_dram, actT, scale, v_dst,
                     brow=None, rec=None, inject=None):
            for js in range(NJ):
                slab = wpool.tile([P, KC, JB], F16, tag="w16", bufs=2,
                                  name="w16")
                nc.sync.dma_start(out=slab, in_=wt_dram[js, :, :, :])
                if inject is not None and js in inject:
                    inject[js]()
                for bt in range(NB):
                    ps = pmm.tile([P, JB], F32, tag="mm", name="mm")
                    first = True
                    if brow is not None:
                        pe.matmul(ps, rec[0:1, bt * P:(bt + 1) * P],
                                  brow[0:1, js * JB:(js + 1) * JB],
                                  start=True, stop=False)
                        first = False
                    for c in range(KC):
                        pe.matmul(
                            ps,
                            actT[:, c, bt * P:(bt + 1) * P],
                            slab[:, c, :],
                            start=(first and c == 0),
                            stop=(c == KC - 1),
                        )
                    sca.activation(
                        out=v_dst[bt][:, js * JB:(js + 1) * JB], in_=ps,
                        func=AF.Copy, scale=scale[:, bt:bt + 1],
                    )

        ones = stile(name="ones", pt=True)
        dve.memset(ones, 1.0)
        n2ab_r = stile(2 * NB, "n2ab_r", pt=True)
        n2ab_z = stile(2 * NB, "n2ab_z", pt=True)
        # pair layout for branch3: [v6_0 v6_1 v5_0 v5_1 | v6_2 v6_3 v5_2 v5_3]
        n2w = stile(2 * NB, "n2w", pt=True)
        d65 = stile(name="d65", pt=True)    # col (bt//2)*2 + bt%2
        d12 = stile(name="d12", pt=True)
        d34 = stile(name="d34", pt=True)
        n2zh = stile(name="n2zh", pt=True)
        dhzh = stile(name="dhzh", pt=True)
        n2q = stile(name="n2q", pt=True)
        dhq = stile(name="dhq", pt=True)
        n2zq = stile(name="n2zq", pt=True)
        dzhzq = stile(name="dzhzq", pt=True)
        dhzq = stile(name="dhzq", pt=True)
        wwslabs = [None] * NJ

        with tc.tile_pool(name="mid", bufs=1) as mid:
            with tc.tile_pool(name="w8p", bufs=1) as w8p:
                with tc.tile_pool(name="xp8", bufs=1) as xp8:
                    with tc.tile_pool(name="early", bufs=1) as early:
                        hxT8 = load_aT(early, hxT8_d, F8, "hxT8")
                        # head: hx load, |hx|^2 -> s_h (gates only G1 copies)
                        nc.sync.dma_start(out=hx_t, in_=hx16_d[:, :, :])
                        # pre-load the sigmoid table set (contains tanh/square/
                        # copy too) so no ACT_TABLE_LOAD lands mid-kernel
                        warm = stile(name="warm", pt=True)
                        sca.activation(warm, ones, AF.Sigmoid)
                        for bt in range(NB):
                            sq_accum_act(hx[bt], n2xh[:, NB + bt:NB + bt + 1])
                        s_h = g_poly(n2xh[:, NB:], pt=True, name="s_h")
                        sc8_h = stile(name="sc8_h", pt=True)
                        dve.tensor_scalar(sc8_h, s_h, 1.0 / RS8, None, OP.mult)
                        # G1: v1 = s_h*(hx @ w_r.T) -> A
                        gemm_fp8(w8p, w8_d["wr8"], hxT8, sc8_h, A)

                        xT8 = load_aT(xp8, xT8_d, F8, "xT8")
                        brz8 = perm.tile([1, 2 * D], F8, tag="brz8",
                                         name="brz8")
                        nc.sync.dma_start(out=brz8, in_=brz8_d[:, :])
                        bw16 = perm.tile([1, D], F16, tag="bw16", name="bw16")
                        nc.sync.dma_start(out=bw16, in_=bw16_d[:, :])
                        # x natural into slot D (dead until v4); |x|^2 -> s_x
                        nc.sync.dma_start(out=D_big, in_=x16_d[:, :, :])
                        for bt in range(NB):
                            sq_accum_dve(Ds[bt], n2xh[:, bt:bt + 1])
                        s_x, sc8_x, recT8, recT5 = make_sx()

                        # G3: v3 = s_h*(hx @ w_z.T) -> B
                        gemm_fp8(w8p, w8_d["wz8"], hxT8, sc8_h, Bs)
                        sq_accum_act(A[0], n2ab_r[:, 0:1])
                        sq_accum_act(A[1], n2ab_r[:, 1:2])
                        sq_accum_act(A[2], n2ab_r[:, 2:3])
                        sq_accum_act(A[3], n2ab_r[:, 3:4])

                    xT16 = load_aT(mid, xT16_d, F16, "xT16")
                    # G2: v2 = s_x*(x @ u_r.T) + br -> C
                    gemm_fp8(w8p, w8_d["ur8"], xT8, sc8_x, C,
                             brow=brz8[:, 0:D], rec=recT8)
                    sq_accum_act(Bs[0], n2ab_z[:, 0:1])
                    sq_accum_act(Bs[1], n2ab_z[:, 1:2])
                    sq_accum_act(Bs[2], n2ab_z[:, 2:3])
                    sq_accum_act(Bs[3], n2ab_z[:, 3:4])

                    # G4: v4 = s_x*(x @ u_z.T) + bz -> D
                    gemm_fp8(w8p, w8_d["uz8"], xT8, sc8_x, Ds,
                             brow=brz8[:, D:2 * D], rec=recT8)
                    sq_accum_act(C[0], n2ab_r[:, 4:5])
                    sq_accum_act(C[1], n2ab_r[:, 5:6])
                    sq_accum_act(C[2], n2ab_r[:, 6:7])
                    sq_accum_act(C[3], n2ab_r[:, 7:8])

            def prefetch_ww0():
                t = ww01p.tile([P, KC, JB], F16, tag="ww0", name="ww0")
                nc.sync.dma_start(out=t, in_=w16_d["ww16"][0, :, :, :])
                wwslabs[0] = t

            # ----- r branch (overlaps G4 on PE); m1 -> A -----
            for bt in range(NB):
                dot_accum(A[bt], C[bt], d12[:, bt:bt + 1])
            ua1, ub1, b1 = branch(n2ab_r, d12, "r")
            for bt in range(NB):
                combine(A[bt], A[bt], ua1[:, bt:bt + 1], C[bt],
                        ub1[:, bt:bt + 1])

            with tc.tile_pool(name="wp16", bufs=1) as wp16:
                # G5: v5 = s_x*(x @ u_w.T) + bw -> C
                gemm_f16(wp16, w16_d["uw16"], xT16, s_x, C,
                         brow=bw16[:, :], rec=recT5,
                         inject={1: prefetch_ww0})

            # sigmoids queue behind G5's copies on ACT (r bounces via junk so
            # slot C is free for v5); p = r * hx -> A overwrites m1
            for bt in range(NB):
                rj = junk()
                sca.activation(rj, A[bt], AF.Sigmoid,
                               scale=b1[:, bt:bt + 1])   # r
                dve.tensor_tensor(A[bt], rj, hx[bt], OP.mult)
            sq_accum_act(Ds[0], n2ab_z[:, 4:5])
            sq_accum_act(Ds[1], n2ab_z[:, 5:6])
            sq_accum_act(Ds[2], n2ab_z[:, 6:7])
            sq_accum_act(Ds[3], n2ab_z[:, 7:8])

            # ----- z branch (overlaps G5; sigmoids queue after G5 copies) ----
            for bt in range(NB):
                dot_accum(Bs[bt], Ds[bt], d34[:, bt:bt + 1])
            ua2, ub2, b2 = branch(n2ab_z, d34, "z")
            for bt in range(NB):
                combine(Bs[bt], Bs[bt], ua2[:, bt:bt + 1], Ds[bt],
                        ub2[:, bt:bt + 1])
                sca.activation(Ds[bt], Bs[bt], AF.Sigmoid,
                               scale=b2[:, bt:bt + 1])       # z -> D

            # ----- transpose p into pT (PE: after G5 mms, before G6 mms) ----
            for bt in range(NB):
                for cp in range(KC // 4):
                    ps = ptr.tile([P, JB], F16, tag="tr", name="tr")
                    for k in range(4):
                        pe.transpose(
                            ps[:, k * P:(k + 1) * P],
                            A[bt][:, (cp * 4 + k) * P:(cp * 4 + k + 1) * P],
                            ident16,
                        )
                    dve.tensor_copy(
                        out=pT[:, cp * 4:cp * 4 + 4, bt * P:(bt + 1) * P],
                        in_=ps.rearrange("p (c b) -> p c b", c=4),
                    )

            # ----- zh = z * hx -> B (+norm +<hx,zh>) -----
            for bt in range(NB):
                dve.tensor_tensor(Bs[bt], Ds[bt], hx[bt], OP.mult)
                sq_accum_act(Bs[bt], n2zh[:, bt:bt + 1])
                dot_accum(hx[bt], Bs[bt], dhzh[:, bt:bt + 1])

        # ----- G6: v6 = s_h*(p @ w.T) -> A, bt-major; tail pair-pipelined ----
        with tc.tile_pool(name="ww23p", bufs=1) as ww23p:
            for js in (1, 2, 3):
                t = ww23p.tile([P, KC, JB], F16, tag=f"ww{js}", name=f"ww{js}")
                nc.sync.dma_start(out=t, in_=w16_d["ww16"][js, :, :, :])
                wwslabs[js] = t

            b3s = [None, None]

            def pair_chain(g):
                """branch3 + m3 for bts (2g, 2g+1); DVE work only."""
                sl = slice(4 * g, 4 * g + 4)
                ua3, ub3, b3 = branch(n2w[:, sl], d65[:, 2 * g:2 * g + 2],
                                      f"w{g}")
                for i, bt in enumerate((2 * g, 2 * g + 1)):
                    dve.tensor_scalar(C[bt], C[bt], ub3[:, i:i + 1], None,
                                      OP.mult)
                    dve.tensor_scalar(A[bt], A[bt], ua3[:, i:i + 1], None,
                                      OP.mult)
                    dve.tensor_tensor(A[bt], A[bt], C[bt], OP.add)  # m3
                b3s[g] = b3

            def pair_tail(g):
                """tanh + squares + dots + zq for bts (2g, 2g+1)."""
                for i, bt in enumerate((2 * g, 2 * g + 1)):
                    sca.activation(C[bt], A[bt], AF.Tanh,
                                   scale=b3s[g][:, i:i + 1])     # q -> C
                    sca.activation(junk(), C[bt], AF.Square,
                                   accum_out=n2q[:, bt:bt + 1])
                    dot_accum_split(hx[bt], C[bt], dhq[:, bt:bt + 1], A[bt])
                    # zq = z*q -> D (z dies)
                    dve.tensor_tensor(Ds[bt], Ds[bt], C[bt], OP.mult)
                    sq_accum_act(Ds[bt], n2zq[:, bt:bt + 1])
                    dot_accum_split(Bs[bt], Ds[bt], dzhzq[:, bt:bt + 1], A[bt])
                    dot_accum_split(hx[bt], Ds[bt], dhzq[:, bt:bt + 1], A[bt])

            for bt in range(NB):
                for js in range(NJ):
                    ps = pmm.tile([P, JB], F32, tag="mm", name="mm")
                    for c in range(KC):
                        pe.matmul(
                            ps,
                            pT[:, c, bt * P:(bt + 1) * P],
                            wwslabs[js][:, c, :],
                            start=(c == 0),
                            stop=(c == KC - 1),
                        )
                    sca.activation(
                        out=A[bt][:, js * JB:(js + 1) * JB], in_=ps,
                        func=AF.Copy, scale=s_h[:, bt:bt + 1],
                    )
                col = (bt // 2) * 4 + (bt % 2)
                sq_accum_act(A[bt], n2w[:, col:col + 1])
                dcol = (bt // 2) * 2 + (bt % 2)
                dot_accum(A[bt], C[bt], d65[:, dcol:dcol + 1])
                if bt == 0:
                    # v5 norms (pair layout), issued late so they never gate
                    # the preceding GEMMs' slab DMAs
                    for b2_ in range(NB):
                        vcol = (b2_ // 2) * 4 + 2 + (b2_ % 2)
                        sq_accum_act(C[b2_], n2w[:, vcol:vcol + 1])
                if bt == 1:
                    pair_chain(0)   # overlaps G6 bts 2,3 on DVE
                    pair_tail(0)
            pair_chain(1)
            pair_tail(1)

            # d-scalars: mobius(-hx, delta*q)
            delta = t_poly(n2q, pt=True, name="delta")
            thq2 = stile(name="thq2", pt=True)
            dve.tensor_tensor(thq2, delta, delta, OP.mult)
            dve.tensor_tensor(thq2, thq2, n2q, OP.mult)
            n2h = n2xh[:, NB:]
            ua_d, ub_d, n2d = mobius(ones, n2h, delta, thq2, dhq,
                                     n2h, n2q, neg_a=True, pfx="d")
            beta_d = g_poly(n2d, pt=True, name="beta_d")

            # e-scalars (e = ua_d*zh + ub_d*zq, never materialized)
            n2e = stile(name="n2e", pt=True)
            t1 = stile(name="te1")
            dve.tensor_tensor(t1, ua_d, ua_d, OP.mult)
            dve.tensor_tensor(t1, t1, n2zh, OP.mult)
            t2 = stile(name="te2")
            dve.tensor_tensor(t2, ub_d, ub_d, OP.mult)
            dve.tensor_tensor(t2, t2, n2zq, OP.mult)
            t3 = stile(name="te3")
            dve.tensor_tensor(t3, ua_d, ub_d, OP.mult)
            dve.tensor_tensor(t3, t3, dzhzq, OP.mult)
            dve.scalar_tensor_tensor(n2e, t3, 2.0, t1, OP.mult, OP.add)
            dve.tensor_tensor(n2e, n2e, t2, OP.add)

            nt2 = stile(name="nt2", pt=True)
            dve.tensor_tensor(nt2, beta_d, beta_d, OP.mult)
            dve.tensor_tensor(nt2, nt2, n2e, OP.mult)
            tt = t_poly(nt2, pt=True, name="tt")
            eps_s = stile(name="eps_s", pt=True)
            dve.tensor_tensor(eps_s, tt, beta_d, OP.mult)
            tht2 = stile(name="tht2", pt=True)
            dve.tensor_tensor(tht2, tt, tt, OP.mult)
            dve.tensor_tensor(tht2, tht2, nt2, OP.mult)
            dhe = stile(name="dhe", pt=True)
            dve.tensor_tensor(dhe, ua_d, dhzh, OP.mult)
            t4 = stile(name="te4")
            dve.tensor_tensor(t4, ub_d, dhzq, OP.mult)
            dve.tensor_tensor(dhe, dhe, t4, OP.add)

            uo, vo, _ = mobius(ones, n2h, eps_s, tht2, dhe, n2h, n2e, pfx="o")
            c_zh = stile(name="c_zh", pt=True)
            dve.tensor_tensor(c_zh, vo, ua_d, OP.mult)
            c_zq = stile(name="c_zq", pt=True)
            dve.tensor_tensor(c_zq, vo, ub_d, OP.mult)

            for bt in range(NB):
                dve.tensor_scalar(Ds[bt], Ds[bt], c_zq[:, bt:bt + 1], None,
                                  OP.mult)
                dve.tensor_scalar(Bs[bt], Bs[bt], c_zh[:, bt:bt + 1], None,
                                  OP.mult)
                dve.tensor_tensor(Ds[bt], Ds[bt], Bs[bt], OP.add)
                o = outt[bt % 2]
                dve.scalar_tensor_tensor(o, hx[bt], uo[:, bt:bt + 1],
                                         Ds[bt], OP.mult, OP.add)
                nc.sync.dma_start(out=out_d[bt * P:(bt + 1) * P, :], in_=o)

    nc.compile()
    return nc


_BUILD_LOCK = threading.Lock()
_NC_CACHE = {}


def _get_nc():
    with _BUILD_LOCK:
        if "nc" not in _NC_CACHE:
            _NC_CACHE["nc"] = _build()
        return _NC_CACHE["nc"]


def _prep_in_maps(inputs):
    f8 = ml_dtypes.float8_e4m3
    x = np.asarray(inputs["x"], dtype=np.float32)
    hx = np.asarray(inputs["hx"], dtype=np.float32)

    def swz_w(a, scale, dt):
        # W [D_out, D_in] -> wT [K=D_in, J=D_out] -> [NJ, P, KC, JB] js-major
        wt = np.asarray(a, np.float32).T * scale
        if dt is f8:
            wt = np.clip(wt, -240, 240)
        return np.ascontiguousarray(
            wt.reshape(KC, P, NJ, JB).transpose(2, 1, 0, 3)
        ).astype(dt)

    def swz_aT(aT, scale, dt):
        # aT [D, BL] -> [P, KC, BL]
        a = aT * scale
        if dt is f8:
            a = np.clip(a, -240, 240)
        return np.ascontiguousarray(
            a.reshape(KC, P, BL).transpose(1, 0, 2)
        ).astype(dt)

    def swz_nat(a):
        # [BL, D] -> [P, NB, D]
        return np.ascontiguousarray(
            np.asarray(a, np.float32).reshape(NB, P, D).transpose(1, 0, 2)
        ).astype(np.float16)

    weights = {
        "wr8": swz_w(inputs["w_r"], SW, f8),
        "ur8": swz_w(inputs["u_r_w"], SW, f8),
        "wz8": swz_w(inputs["w_z"], SW, f8),
        "uz8": swz_w(inputs["u_z_w"], SW, f8),
        "uw16": swz_w(inputs["u_w"], 1.0, np.float16),
        "ww16": swz_w(inputs["w"], 1.0, np.float16),
    }
    brz = np.concatenate([
        np.asarray(inputs["u_r_b"], np.float32),
        np.asarray(inputs["u_z_b"], np.float32),
    ]).reshape(1, 2 * D) * SB8
    biases = {
        "brz8": np.clip(brz, -240, 240).astype(f8),
        "bw16": np.asarray(inputs["u_b"], np.float32).reshape(1, D).astype(
            np.float16),
    }

    in_maps = []
    for c in range(N_CORES):
        xs = x[c * BL:(c + 1) * BL]
        hs = hx[c * BL:(c + 1) * BL]
        xsT = np.ascontiguousarray(xs.T)
        hsT = np.ascontiguousarray(hs.T)
        m = {
            "x16": swz_nat(xs),
            "hx16": swz_nat(hs),
            "xT8": swz_aT(xsT, SA, f8),
            "hxT8": swz_aT(hsT, SA, f8),
            "xT16": swz_aT(xsT, 1.0, np.float16),
        }
        m.update(weights)
        m.update(biases)
        in_maps.append(m)
    return in_maps


def kernel(**inputs: np.ndarray) -> np.ndarray:
    in_maps = _prep_in_maps(inputs)
    nc = _get_nc()
    res = run_bass_kernel_spmd(nc, in_maps, core_ids=list(range(N_CORES)))
    return np.concatenate([r["out"] for r in res.results], axis=0)



# revision 19
# speedup vs baseline: 1.0330x; 1.0330x over previous
"""Trainium2 Bass kernel for the hyperbolic (Poincare-ball) GRU cell.

Data-parallel over batch across 8 NeuronCores, no collectives.

v2 schedule (vs the 392us baseline):
  - GEMM order G1,G2,G3,G4 (fp8 DoubleRow), G5,G6 (fp16; fp8 fails the
    2e-2 gate - sim'd 1.6-2.7e-2), G6 bt-major so each batch-tile's tail
    chain starts as soon as its v6 lands.
  - Rank-1 bias matmuls removed: biases live in SBUF as partition-broadcast
    tiles (built once via rank-1 PE matmuls at head) and are added by the
    DVE copy (scalar_tensor_tensor: psum*scale + bias).
  - PSUM->SBUF copies split by engine: unbiased GEMMs (G1,G3,G6) copy on
    ACT (scale=col), biased (G2,G4,G5) on DVE stt.
  - Reductions balanced ACT (Square+accum) / DVE (stt+accum, 1x);
    dhq computed via the |hx+q|^2 sum-identity to shift work to ACT.
  - dhzq = <hx,z*q> = <zh,q>, dzhzq = <zh,zq>: reuse zh to start these
    immediately after q.
  - d/e mobius scalar chains + final combines run per bt-PAIR so pair 0
    finishes during G6.
  - Head DMA priority: hxT8 + wr8 slab0 first -> first MM ~{6,7}us.
"""

import threading
from contextlib import ExitStack

import ml_dtypes
import numpy as np

import concourse.bacc as bacc
import concourse.mybir as mybir
import concourse.tile as tile
from concourse.bass_utils import run_bass_kernel_spmd
from concourse.masks import make_identity

F32 = mybir.dt.float32
F16 = mybir.dt.float16
F8 = mybir.dt.float8e4
AF = mybir.ActivationFunctionType
OP = mybir.AluOpType
PM = mybir.MatmulPerfMode

N_CORES = 8
B, D = 4096, 2048
BL = B // N_CORES          # 512 rows per core
P = 128                    # partitions
NB = BL // P               # 4 batch tiles per core
KC = D // P                # 16 contraction chunks
JB = 512                   # GEMM j-block width
NJ = D // JB               # 4 j-blocks

SA = 2048.0                # fp8 activation scale
SW = 1024.0                # fp8 weight scale
RS8 = SA * SW              # folded out of fp8 GEMM copies

EPS = 1e-5

T_COEF = [1.0, -0.333333329, 0.133333183, -0.05396619044, 0.02185495027,
          -0.008803496923, 0.003439097315, -0.001203079678, 0.0003169332143,
          -4.391237846e-05]
G_COEF = [0.9986986426, 0.3781016454, -0.4412363167, 5.173515875, -23.8123998,
          72.00036756, -137.3859594, 162.2957814, -108.0106185, 31.34506744]


def _build():
    nc = bacc.Bacc(None, target_bir_lowering=False, debug=False)

    hx16_d = nc.dram_tensor("hx16", [P, NB, D], F16, kind="ExternalInput")
    x16_d = nc.dram_tensor("x16", [P, NB, D], F16, kind="ExternalInput")
    xT8_d = nc.dram_tensor("xT8", [P, KC, BL], F8, kind="ExternalInput")
    hxT8_d = nc.dram_tensor("hxT8", [P, KC, BL], F8, kind="ExternalInput")
    xT16_d = nc.dram_tensor("xT16", [P, KC, BL], F16, kind="ExternalInput")
    w8_d = {
        name: nc.dram_tensor(name, [NJ, P, KC, JB], F8, kind="ExternalInput")
        for name in ["wr8", "wz8", "ur8", "uz8"]
    }
    w16_d = {
        name: nc.dram_tensor(name, [NJ, P, KC, JB], F16, kind="ExternalInput")
        for name in ["uw16", "ww16"]
    }
    brzw_d = nc.dram_tensor("brzw16", [1, 3 * D], F16, kind="ExternalInput")
    out_d = nc.dram_tensor("out", [BL, D], F32, kind="ExternalOutput")

    with ExitStack() as ctx:
        tc = ctx.enter_context(tile.TileContext(nc))
        dve, sca, pe = nc.vector, nc.scalar, nc.tensor

        # ---------------- persistent pools ----------------
        scal = ctx.enter_context(tc.tile_pool(name="scal", bufs=1))
        slot = ctx.enter_context(tc.tile_pool(name="slot", bufs=1))
        hxp = ctx.enter_context(tc.tile_pool(name="hxp", bufs=1))
        ptp = ctx.enter_context(tc.tile_pool(name="ptp", bufs=1))
        junkp = ctx.enter_context(tc.tile_pool(name="junkp", bufs=2))
        biasp = ctx.enter_context(tc.tile_pool(name="biasp", bufs=1))
        pmm = ctx.enter_context(tc.tile_pool(name="pmm", bufs=6, space="PSUM"))
        ptr = ctx.enter_context(tc.tile_pool(name="ptr", bufs=2, space="PSUM"))

        def stile(w=NB, name="s", pt=False):
            if pt:
                return scal.tile([P, w], F32, tag=f"P_{name}", bufs=1, name=name)
            return scal.tile([P, w], F32, tag=f"scal{w}",
                             bufs=(64 if w <= NB else 16), name=name)

        def junk():
            return junkp.tile([P, D], F16, tag="junk", bufs=2, name="junk")

        def slot_group(nm):
            t = slot.tile([P, NB, D], F16, tag=nm, name=nm)
            return t, [t[:, bt, :] for bt in range(NB)]

        A_big, A = slot_group("A")
        B_big, Bs = slot_group("B")
        C_big, C = slot_group("C")
        D_big, Ds = slot_group("D")
        hx_t = hxp.tile([P, NB, D], F16, tag="hx", name="hx")
        hx = [hx_t[:, bt, :] for bt in range(NB)]
        pT = ptp.tile([P, KC, BL], F16, tag="pT", name="pT")
        # bias broadcast tiles: [P, 2D] for br|bz, [P, D] for bw
        brz_bc = biasp.tile([P, 2 * D], F16, tag="brz_bc", name="brz_bc")
        bw_bc = biasp.tile([P, D], F16, tag="bw_bc", name="bw_bc")

        # ---------------- tiny-op helpers ----------------
        def sq_accum_dve(t, acc_col):
            dve.scalar_tensor_tensor(junk(), t, 1.0, t, OP.mult, OP.mult,
                                     accum_out=acc_col)

        def sq_accum_act(t, acc_col):
            sca.activation(junk(), t, AF.Square, accum_out=acc_col)

        def dot_accum(a, b, acc_col):
            dve.scalar_tensor_tensor(junk(), a, 1.0, b, OP.mult, OP.mult,
                                     accum_out=acc_col)

        def poly(dst, u, coefs):
            dve.tensor_scalar(dst, u, float(coefs[-1]), float(coefs[-2]),
                              OP.mult, OP.add)
            for ck in reversed(coefs[:-2]):
                dve.tensor_tensor(dst, dst, u, OP.mult)
                dve.tensor_scalar(dst, dst, float(ck), None, OP.add)

        def t_poly(u, w=NB, pt=False, name="tp"):
            uc = stile(w, "uc")
            dve.tensor_scalar(uc, u, 0.95, None, OP.min)
            o = stile(w, name, pt=pt)
            poly(o, uc, T_COEF)
            return o

        def g_poly(u, w=NB, pt=False, name="gp"):
            uc = stile(w, "ug")
            dve.tensor_scalar(uc, u, 0.10, 0.80, OP.max, OP.min)
            o = stile(w, name, pt=pt)
            poly(o, uc, G_COEF)
            return o

        def mobius(al_a, x2, al_b, y2, dab, n2a, n2b, neg_a=False, pfx="m"):
            """Returns ua, ub, n2m = |ua*a + ub*b|^2 (width from dab)."""
            w = dab.shape[-1]
            xy = stile(w, name="xy")
            dve.tensor_tensor(xy, al_a, al_b, OP.mult)
            dve.tensor_tensor(xy, xy, dab, OP.mult)
            if neg_a:
                dve.tensor_scalar(xy, xy, -1.0, None, OP.mult)
            txy1 = stile(w, name="txy1")
            dve.tensor_scalar(txy1, xy, 2.0, 1.0, OP.mult, OP.add)
            numa = stile(w, name="numa")
            dve.tensor_tensor(numa, txy1, y2, OP.add)
            den = stile(w, name="den")
            dve.tensor_tensor(den, x2, y2, OP.mult)
            dve.tensor_tensor(den, den, txy1, OP.add)
            dve.tensor_scalar(den, den, float(EPS), None, OP.max)
            rden = stile(w, name="rden")
            dve.reciprocal(rden, den)
            ua = stile(w, name=f"{pfx}_ua", pt=True)
            dve.tensor_tensor(ua, numa, al_a, OP.mult)
            dve.tensor_tensor(ua, ua, rden, OP.mult)
            if neg_a:
                dve.tensor_scalar(ua, ua, -1.0, None, OP.mult)
            ub = stile(w, name=f"{pfx}_ub", pt=True)
            dve.tensor_scalar(ub, x2, -1.0, 1.0, OP.mult, OP.add)
            dve.tensor_tensor(ub, ub, al_b, OP.mult)
            dve.tensor_tensor(ub, ub, rden, OP.mult)
            t1 = stile(w, name="t1")
            dve.tensor_tensor(t1, ua, ua, OP.mult)
            dve.tensor_tensor(t1, t1, n2a, OP.mult)
            t2 = stile(w, name="t2")
            dve.tensor_tensor(t2, ub, ub, OP.mult)
            dve.tensor_tensor(t2, t2, n2b, OP.mult)
            t3 = stile(w, name="t3")
            dve.tensor_tensor(t3, ua, ub, OP.mult)
            dve.tensor_tensor(t3, t3, dab, OP.mult)
            n2m = stile(w, name=f"{pfx}_n2m", pt=True)
            dve.scalar_tensor_tensor(n2m, t3, 2.0, t1, OP.mult, OP.add)
            dve.tensor_tensor(n2m, n2m, t2, OP.add)
            return ua, ub, n2m

        def branch(n2ab, dab, pfx):
            """n2ab: [P, 2w] (first half |a|^2, second |b|^2) -> ua, ub, beta."""
            w = dab.shape[-1]
            al = t_poly(n2ab, w=2 * w)
            x2y2 = stile(2 * w, "x2y2")
            dve.tensor_tensor(x2y2, al, al, OP.mult)
            dve.tensor_tensor(x2y2, x2y2, n2ab, OP.mult)
            ua, ub, n2m = mobius(al[:, 0:w], x2y2[:, 0:w],
                                 al[:, w:], x2y2[:, w:],
                                 dab, n2ab[:, 0:w], n2ab[:, w:], pfx=pfx)
            return ua, ub, g_poly(n2m, w=w, pt=True, name=f"{pfx}_beta")

        def combine(dst, a, ua_col, b, ub_col):
            """dst = ua*a + ub*b; b is scaled in place."""
            dve.tensor_scalar(b, b, ub_col, None, OP.mult)
            dve.scalar_tensor_tensor(dst, a, ua_col, b, OP.mult, OP.add)

        # persistent scalar columns
        n2xh = stile(2 * NB, "n2xh", pt=True)   # cols 0..3 |x|^2, 4..7 |hx|^2
        ones = stile(name="ones", pt=True)
        n2ab_r = stile(2 * NB, "n2ab_r", pt=True)
        n2ab_z = stile(2 * NB, "n2ab_z", pt=True)
        n2w = stile(2 * NB, "n2w", pt=True)   # [v6_0 v6_1 v5_0 v5_1 | ...]
        d12 = stile(name="d12", pt=True)
        d34 = stile(name="d34", pt=True)
        d65 = stile(name="d65", pt=True)      # col (bt//2)*2 + bt%2
        n2zh = stile(name="n2zh", pt=True)
        dhzh = stile(name="dhzh", pt=True)
        n2q = stile(name="n2q", pt=True)
        dhq = stile(name="dhq", pt=True)
        n2s = stile(name="n2s", pt=True)      # |hx+q|^2 per bt
        n2zq = stile(name="n2zq", pt=True)
        dzhzq = stile(name="dzhzq", pt=True)
        dhzq = stile(name="dhzq", pt=True)
        uo_a = stile(name="uo_a", pt=True)
        czh_a = stile(name="czh_a", pt=True)
        czq_a = stile(name="czq_a", pt=True)
        wwslabs = [None] * NJ

        # ---------------- GEMM machinery ----------------
        def load_aT(pool, src, dtype, nm):
            t = pool.tile([P, KC, BL], dtype, tag=f"aT_{nm}", name=nm)
            nc.sync.dma_start(out=t, in_=src[:, :, :])
            return t

        def gemm_fp8(wpool, wt_dram, actT, scale, v_dst,
                     bias_bc=None, bias_off=0, inject=None, pre=None):
            """fp8 DoubleRow GEMM. bias_bc None -> ACT copies (scale col);
            else DVE stt copies (psum*scale + bias)."""
            for js in range(NJ):
                if pre is not None and js == 0:
                    slab = pre
                else:
                    slab = wpool.tile([P, KC, JB], F8, tag="w8", bufs=2,
                                      name="w8")
                    nc.sync.dma_start(out=slab, in_=wt_dram[js, :, :, :])
                if inject is not None and js in inject:
                    inject[js]()
                for bt in range(NB):
                    ps = pmm.tile([P, JB], F32, tag="mm", name="mm")
                    for c in range(KC // 2):
                        pe.matmul(
                            ps,
                            actT[:, 2 * c:2 * c + 2, bt * P:(bt + 1) * P],
                            slab[:, 2 * c:2 * c + 2, :],
                            start=(c == 0),
                            stop=(c == KC // 2 - 1),
                            perf_mode=PM.DoubleRow,
                        )
                    dst = v_dst[bt][:, js * JB:(js + 1) * JB]
                    if bias_bc is None:
                        sca.activation(out=dst, in_=ps, func=AF.Copy,
                                       scale=scale[:, bt:bt + 1])
                    else:
                        off = bias_off + js * JB
                        dve.scalar_tensor_tensor(
                            dst, ps, scale[:, bt:bt + 1],
                            bias_bc[:, off:off + JB], OP.mult, OP.add)

        def gemm_f16(wpool, wt_dram, actT, scale, v_dst,
                     bias_bc=None, bias_off=0, inject=None):
            for js in range(NJ):
                slab = wpool.tile([P, KC, JB], F16, tag="w16", bufs=2,
                                  name="w16")
                nc.sync.dma_start(out=slab, in_=wt_dram[js, :, :, :])
                if inject is not None and js in inject:
                    inject[js]()
                for bt in range(NB):
                    ps = pmm.tile([P, JB], F32, tag="mm", name="mm")
                    for c in range(KC):
                        pe.matmul(
                            ps,
                            actT[:, c, bt * P:(bt + 1) * P],
                            slab[:, c, :],
                            start=(c == 0),
                            stop=(c == KC - 1),
                        )
                    dst = v_dst[bt][:, js * JB:(js + 1) * JB]
                    if bias_bc is None:
                        sca.activation(out=dst, in_=ps, func=AF.Copy,
                                       scale=scale[:, bt:bt + 1])
                    else:
                        off = bias_off + js * JB
                        dve.scalar_tensor_tensor(
                            dst, ps, scale[:, bt:bt + 1],
                            bias_bc[:, off:off + JB], OP.mult, OP.add)

        # =============== head ===============
        phaseAB = ctx.enter_context(ExitStack())
        wwpB = ctx.enter_context(tc.tile_pool(name="wwpB", bufs=1))
        for js in (0, 1):
            wwslabs[js] = wwpB.tile([P, KC, JB], F16, tag=f"ww{js}",
                                    name=f"ww{js}")
        xp16 = phaseAB.enter_context(tc.tile_pool(name="xp16", bufs=1))
        xT16 = xp16.tile([P, KC, BL], F16, tag="aT_xT16", name="xT16")
        with ExitStack() as phaseA:
            early = phaseA.enter_context(tc.tile_pool(name="early", bufs=1))
            w8p = phaseA.enter_context(tc.tile_pool(name="w8p", bufs=1))
            xp8 = phaseA.enter_context(tc.tile_pool(name="xp8", bufs=1))

            ident16 = early.tile([P, P], F16, tag="id16", name="id16")
            make_identity(nc, ident16)

            # DMA priority: hxT8, wr8 slab0, hx16, bias rows
            hxT8 = load_aT(early, hxT8_d, F8, "hxT8")
            wr_s0 = w8p.tile([P, KC, JB], F8, tag="w8", bufs=2, name="w8")
            nc.sync.dma_start(out=wr_s0, in_=w8_d["wr8"][0, :, :, :])
            nc.sync.dma_start(out=hx_t, in_=hx16_d[:, :, :])
            # bias rows partition-broadcast into SBUF
            nc.sync.dma_start(
                out=brz_bc, in_=brzw_d[0:1, 0:2 * D].partition_broadcast(P))
            nc.sync.dma_start(
                out=bw_bc, in_=brzw_d[0:1, 2 * D:3 * D].partition_broadcast(P))

            # warm the sigmoid table set (has tanh/square/copy too)
            dve.memset(ones, 1.0)
            warm = stile(name="warm", pt=True)
            sca.activation(warm, ones, AF.Sigmoid)

            # |hx|^2 -> s_h (gates G1 copies)
            for bt in range(NB):
                sq_accum_act(hx[bt], n2xh[:, NB + bt:NB + bt + 1])
            s_h = g_poly(n2xh[:, NB:], pt=True, name="s_h")
            sc8_h = stile(name="sc8_h", pt=True)
            dve.tensor_scalar(sc8_h, s_h, 1.0 / RS8, None, OP.mult)

            # ---- G1: v1 = s_h*(hx @ w_r.T) -> A (ACT copies) ----
            gemm_fp8(w8p, w8_d["wr8"], hxT8, sc8_h, A, pre=wr_s0)

            # x loads + |x|^2 -> s_x (needed by G2 copies)
            nc.sync.dma_start(out=D_big, in_=x16_d[:, :, :])
            xT8 = load_aT(xp8, xT8_d, F8, "xT8")
            for bt in range(NB):
                sq_accum_dve(Ds[bt], n2xh[:, bt:bt + 1])
            s_x = g_poly(n2xh[:, 0:NB], pt=True, name="s_x")
            sc8_x = stile(name="sc8_x", pt=True)
            dve.tensor_scalar(sc8_x, s_x, 1.0 / RS8, None, OP.mult)

            # |v1|^2 (ACT, after G1 copies)
            for bt in range(NB):
                sq_accum_act(A[bt], n2ab_r[:, bt:bt + 1])

            # ---- G2: v2 = s_x*(x @ u_r.T) + br -> C (DVE stt copies) ----
            gemm_fp8(w8p, w8_d["ur8"], xT8, sc8_x, C,
                     bias_bc=brz_bc, bias_off=0)
            for bt in range(NB):
                sq_accum_dve(C[bt], n2ab_r[:, NB + bt:NB + bt + 1])
                dot_accum(A[bt], C[bt], d12[:, bt:bt + 1])

            # ---- r branch -> m1 -> r -> p (overlaps G3) ----
            ua1, ub1, b1 = branch(n2ab_r, d12, "r")
            for bt in range(NB):
                combine(A[bt], A[bt], ua1[:, bt:bt + 1], C[bt],
                        ub1[:, bt:bt + 1])

            # ---- G3: v3 = s_h*(hx @ w_z.T) -> B (ACT copies) ----
            def g3_post():
                # r sigmoid + p = r*hx, issued into G3's window
                for bt in range(NB):
                    rj = junk()
                    sca.activation(rj, A[bt], AF.Sigmoid,
                                   scale=b1[:, bt:bt + 1])
                    dve.tensor_tensor(A[bt], rj, hx[bt], OP.mult)  # p -> A

            gemm_fp8(w8p, w8_d["wz8"], hxT8, sc8_h, Bs,
                     inject={2: g3_post})
            for bt in range(NB):
                sq_accum_act(Bs[bt], n2ab_z[:, bt:bt + 1])

            # xT16 for G5 (DMA after fp8 weights; tile pre-created)
            nc.sync.dma_start(out=xT16, in_=xT16_d[:, :, :])

            # ---- G4: v4 = s_x*(x @ u_z.T) + bz -> D (DVE stt copies) ----
            gemm_fp8(w8p, w8_d["uz8"], xT8, sc8_x, Ds,
                     bias_bc=brz_bc, bias_off=D)
            for bt in range(NB):
                sq_accum_dve(Ds[bt], n2ab_z[:, NB + bt:NB + bt + 1])
                dot_accum(Bs[bt], Ds[bt], d34[:, bt:bt + 1])

            # ---- z branch -> m2 -> z (overlaps transposes/G5) ----
            ua2, ub2, b2 = branch(n2ab_z, d34, "z")
            for bt in range(NB):
                combine(Bs[bt], Bs[bt], ua2[:, bt:bt + 1], Ds[bt],
                        ub2[:, bt:bt + 1])
                sca.activation(Ds[bt], Bs[bt], AF.Sigmoid,
                               scale=b2[:, bt:bt + 1])       # z -> D

            # ---- transpose p (A) -> pT on PE ----
            for bt in range(NB):
                for cp in range(KC // 4):
                    ps = ptr.tile([P, JB], F16, tag="tr", name="tr")
                    for k in range(4):
                        pe.transpose(
                            ps[:, k * P:(k + 1) * P],
                            A[bt][:, (cp * 4 + k) * P:(cp * 4 + k + 1) * P],
                            ident16,
                        )
                    dve.tensor_copy(
                        out=pT[:, cp * 4:cp * 4 + 4, bt * P:(bt + 1) * P],
                        in_=ps.rearrange("p (c b) -> p c b", c=4),
                    )

        # =============== phase B: G5 + zh work ===============
        with ExitStack() as phaseB:
            wp16 = phaseB.enter_context(tc.tile_pool(name="wp16", bufs=1))

            def prefetch_ww(js, pool):
                def go():
                    if wwslabs[js] is None:
                        wwslabs[js] = pool.tile([P, KC, JB], F16,
                                                tag=f"ww{js}", name=f"ww{js}")
                    nc.sync.dma_start(out=wwslabs[js],
                                      in_=w16_d["ww16"][js, :, :, :])
                return go

            # zh = z*hx -> B; |zh|^2 (ACT); <hx,zh> (DVE) - during G5
            def zh_work():
                for bt in range(NB):
                    dve.tensor_tensor(Bs[bt], Ds[bt], hx[bt], OP.mult)
                    sq_accum_act(Bs[bt], n2zh[:, bt:bt + 1])
                    dot_accum(hx[bt], Bs[bt], dhzh[:, bt:bt + 1])

            # G5: v5 = s_x*(x @ u_w.T) + bw -> C (DVE stt copies)
            gemm_f16(wp16, w16_d["uw16"], xT16, s_x, C,
                     bias_bc=bw_bc, bias_off=0,
                     inject={0: prefetch_ww(0, wwpB), 1: zh_work,
                             2: prefetch_ww(1, wwpB)})
            # |v5|^2 -> n2w pair layout cols (ACT)
            for bt in range(NB):
                vcol = (bt // 2) * 4 + 2 + (bt % 2)
                sq_accum_act(C[bt], n2w[:, vcol:vcol + 1])
        phaseAB.close()

        # =============== phase C: G6 bt-major + tail ===============
        with ExitStack() as phaseC:
            outp = phaseC.enter_context(tc.tile_pool(name="outp", bufs=1))
            outt = [outp.tile([P, D], F32, tag=f"o{i}", name=f"o{i}")
                    for i in range(2)]
            wwpC = phaseC.enter_context(tc.tile_pool(name="wwpC", bufs=1))
            prefetch_ww(2, wwpC)()
            prefetch_ww(3, wwpC)()

            b3s = [None, None]

            def pair_chain(g):
                """branch3 + m3 for bts (2g, 2g+1); DVE work only."""
                sl = slice(4 * g, 4 * g + 4)
                ua3, ub3, b3 = branch(n2w[:, sl], d65[:, 2 * g:2 * g + 2],
                                      f"w{g}")
                for i, bt in enumerate((2 * g, 2 * g + 1)):
                    combine(A[bt], A[bt], ua3[:, i:i + 1], C[bt],
                            ub3[:, i:i + 1])    # m3 -> A (v5 in C scaled)
                b3s[g] = b3

            def pair_tanh(g):
                for i, bt in enumerate((2 * g, 2 * g + 1)):
                    sca.activation(C[bt], A[bt], AF.Tanh,
                                   scale=b3s[g][:, i:i + 1])     # q -> C

            def pair_postq_dve(g):
                for bt in (2 * g, 2 * g + 1):
                    # s = hx + q -> A[bt] (m3 is dead); dhq via sum identity
                    dve.tensor_tensor(A[bt], hx[bt], C[bt], OP.add)
                    sca.activation(junk(), A[bt], AF.Square,
                                   accum_out=n2s[:, bt:bt + 1])
                    # zq = z*q -> D (z dies)
                    dve.tensor_tensor(Ds[bt], Ds[bt], C[bt], OP.mult)
                    # dhzq = <zh, q>
                    dot_accum(Bs[bt], C[bt], dhzq[:, bt:bt + 1])
                    # dzhzq = <zh, zq>
                    dot_accum(Bs[bt], Ds[bt], dzhzq[:, bt:bt + 1])

            def pair_postq_act(g):
                for bt in (2 * g, 2 * g + 1):
                    sq_accum_act(C[bt], n2q[:, bt:bt + 1])
                    sq_accum_act(Ds[bt], n2zq[:, bt:bt + 1])

            def tail_scalars(g):
                """d/e mobius scalar chain for pair g -> uo, c_zh, c_zq."""
                sl = slice(2 * g, 2 * g + 2)
                # dhq = 0.5*(|hx+q|^2 - |hx|^2 - |q|^2)
                dq = dhq[:, sl]
                dve.tensor_tensor(dq, n2s[:, sl], n2q[:, sl], OP.subtract)
                n2h = n2xh[:, NB + 2 * g:NB + 2 * g + 2]
                dve.tensor_tensor(dq, dq, n2h, OP.subtract)
                dve.tensor_scalar(dq, dq, 0.5, None, OP.mult)

                delta = t_poly(n2q[:, sl], w=2, name=f"delta{g}")
                thq2 = stile(2, name="thq2")
                dve.tensor_tensor(thq2, delta, delta, OP.mult)
                dve.tensor_tensor(thq2, thq2, n2q[:, sl], OP.mult)
                ua_d, ub_d, n2d = mobius(ones[:, sl], n2h, delta, thq2,
                                         dq, n2h, n2q[:, sl],
                                         neg_a=True, pfx=f"d{g}")
                beta_d = g_poly(n2d, w=2, name=f"beta_d{g}")

                n2e = stile(2, name="n2e")
                t1 = stile(2, name="te1")
                dve.tensor_tensor(t1, ua_d, ua_d, OP.mult)
                dve.tensor_tensor(t1, t1, n2zh[:, sl], OP.mult)
                t2 = stile(2, name="te2")
                dve.tensor_tensor(t2, ub_d, ub_d, OP.mult)
                dve.tensor_tensor(t2, t2, n2zq[:, sl], OP.mult)
                t3 = stile(2, name="te3")
                dve.tensor_tensor(t3, ua_d, ub_d, OP.mult)
                dve.tensor_tensor(t3, t3, dzhzq[:, sl], OP.mult)
                dve.scalar_tensor_tensor(n2e, t3, 2.0, t1, OP.mult, OP.add)
                dve.tensor_tensor(n2e, n2e, t2, OP.add)

                nt2 = stile(2, name="nt2")
                dve.tensor_tensor(nt2, beta_d, beta_d, OP.mult)
                dve.tensor_tensor(nt2, nt2, n2e, OP.mult)
                tt = t_poly(nt2, w=2, name=f"tt{g}")
                eps_s = stile(2, name="eps_s")
                dve.tensor_tensor(eps_s, tt, beta_d, OP.mult)
                tht2 = stile(2, name="tht2")
                dve.tensor_tensor(tht2, tt, tt, OP.mult)
                dve.tensor_tensor(tht2, tht2, nt2, OP.mult)
                dhe = stile(2, name="dhe")
                dve.tensor_tensor(dhe, ua_d, dhzh[:, sl], OP.mult)
                t4 = stile(2, name="te4")
                dve.tensor_tensor(t4, ub_d, dhzq[:, sl], OP.mult)
                dve.tensor_tensor(dhe, dhe, t4, OP.add)

                uo, vo, _ = mobius(ones[:, sl], n2h, eps_s, tht2, dhe,
                                   n2h, n2e, pfx=f"o{g}")
                dve.tensor_copy(out=uo_a[:, sl], in_=uo)
                dve.tensor_tensor(czh_a[:, sl], vo, ua_d, OP.mult)
                dve.tensor_tensor(czq_a[:, sl], vo, ub_d, OP.mult)

            def final_bt(bt):
                dve.tensor_scalar(Ds[bt], Ds[bt], czq_a[:, bt:bt + 1], None,
                                  OP.mult)
                dve.scalar_tensor_tensor(Ds[bt], Bs[bt], czh_a[:, bt:bt + 1],
                                         Ds[bt], OP.mult, OP.add)
                o = outt[bt % 2]
                dve.scalar_tensor_tensor(o, hx[bt], uo_a[:, bt:bt + 1],
                                         Ds[bt], OP.mult, OP.add)
                nc.sync.dma_start(out=out_d[bt * P:(bt + 1) * P, :], in_=o)

            # ---- G6 bt-major: v6 = s_h*(p @ w.T) -> A ----
            for bt in range(NB):
                for js in range(NJ):
                    ps = pmm.tile([P, JB], F32, tag="mm", name="mm")
                    for c in range(KC):
                        pe.matmul(
                            ps,
                            pT[:, c, bt * P:(bt + 1) * P],
                            wwslabs[js][:, c, :],
                            start=(c == 0),
                            stop=(c == KC - 1),
                        )
                    sca.activation(
                        out=A[bt][:, js * JB:(js + 1) * JB], in_=ps,
                        func=AF.Copy, scale=s_h[:, bt:bt + 1],
                    )
                col = (bt // 2) * 4 + (bt % 2)
                sq_accum_act(A[bt], n2w[:, col:col + 1])   # ACT: |v6|^2
                if bt == 3:
                    # pair0 epilogue: ready DVE work ahead of the d65-bt3 wait
                    tail_scalars(0)
                    final_bt(0)
                    final_bt(1)
                dcol = (bt // 2) * 2 + (bt % 2)
                dot_accum(A[bt], C[bt], d65[:, dcol:dcol + 1])  # DVE
                if bt == 1:
                    pair_chain(0)
                if bt == 2:
                    pair_tanh(0)
                    pair_postq_dve(0)
                    pair_postq_act(0)
            pair_chain(1)
            pair_tanh(1)
            pair_postq_dve(1)
            pair_postq_act(1)
            tail_scalars(1)
            final_bt(2)
            final_bt(3)

    nc.compile()
    return nc


_BUILD_LOCK = threading.Lock()
_NC_CACHE = {}


def _get_nc():
    with _BUILD_LOCK:
        if "nc" not in _NC_CACHE:
            _NC_CACHE["nc"] = _build()
        return _NC_CACHE["nc"]


def _prep_in_maps(inputs):
    f8 = ml_dtypes.float8_e4m3
    x = np.asarray(inputs["x"], dtype=np.float32)
    hx = np.asarray(inputs["hx"], dtype=np.float32)

    def swz_w(a, scale, dt):
        # W [D_out, D_in] -> wT [K=D_in, J=D_out] -> [NJ, P, KC, JB] js-major
        wt = np.asarray(a, np.float32).T * scale
        if dt is f8:
            wt = np.clip(wt, -240, 240)
        return np.ascontiguousarray(
            wt.reshape(KC, P, NJ, JB).transpose(2, 1, 0, 3)
        ).astype(dt)

    def swz_aT(aT, scale, dt):
        # aT [D, BL] -> [P, KC, BL]
        a = aT * scale
        if dt is f8:
            a = np.clip(a, -240, 240)
        return np.ascontiguousarray(
            a.reshape(KC, P, BL).transpose(1, 0, 2)
        ).astype(dt)

    def swz_nat(a):
        # [BL, D] -> [P, NB, D]
        return np.ascontiguousarray(
            np.asarray(a, np.float32).reshape(NB, P, D).transpose(1, 0, 2)
        ).astype(np.float16)

    weights = {
        "wr8": swz_w(inputs["w_r"], SW, f8),
        "ur8": swz_w(inputs["u_r_w"], SW, f8),
        "wz8": swz_w(inputs["w_z"], SW, f8),
        "uz8": swz_w(inputs["u_z_w"], SW, f8),
        "uw16": swz_w(inputs["u_w"], 1.0, np.float16),
        "ww16": swz_w(inputs["w"], 1.0, np.float16),
    }
    brzw = np.concatenate([
        np.asarray(inputs["u_r_b"], np.float32),
        np.asarray(inputs["u_z_b"], np.float32),
        np.asarray(inputs["u_b"], np.float32),
    ]).reshape(1, 3 * D)
    biases = {"brzw16": brzw.astype(np.float16)}

    in_maps = []
    for c in range(N_CORES):
        xs = x[c * BL:(c + 1) * BL]
        hs = hx[c * BL:(c + 1) * BL]
        xsT = np.ascontiguousarray(xs.T)
        hsT = np.ascontiguousarray(hs.T)
        m = {
            "x16": swz_nat(xs),
            "hx16": swz_nat(hs),
            "xT8": swz_aT(xsT, SA, f8),
            "hxT8": swz_aT(hsT, SA, f8),
            "xT16": swz_aT(xsT, 1.0, np.float16),
        }
        m.update(weights)
        m.update(biases)
        in_maps.append(m)
    return in_maps


def kernel(**inputs: np.ndarray) -> np.ndarray:
    in_maps = _prep_in_maps(inputs)
    nc = _get_nc()
    res = run_bass_kernel_spmd(nc, in_maps, core_ids=list(range(N_CORES)))
    return np.concatenate([r["out"] for r in res.results], axis=0)


# revision 25
# speedup vs baseline: 1.0589x; 1.0251x over previous
"""Trainium2 Bass kernel for the hyperbolic (Poincare-ball) GRU cell.

Data-parallel over batch across 8 NeuronCores, no collectives.

v3 schedule (vs 392us baseline / 380us v2):
  - GEMMs G1-G4 fp8 DoubleRow, G5/G6 fp16 (fp8 fails the 2e-2 gate).
    G6 bt-major with all 4 weight slabs resident so each batch-tile's
    tail chain starts as soon as its v6 lands.
  - Rank-1 bias matmuls removed: biases live as partition-broadcast SBUF
    tiles, added by the DVE copy (stt: psum*scale + bias).
  - Engine balance (measured costs: DVE stt 2.26us full-width ALWAYS, TT
    1.14, TS 0.6, ACT pass 1.93): squares on ACT; dots d12/d34/dhzh/dhq
    via |a+b|^2 sum-identity (DVE add + ACT square), d65/dzhzq/dhzq as
    DVE stt; m1/m2 as ts+ts+add on DVE, m3 scale-halves on ACT.
  - SBUF: left/right pool stacks; A-slot split into a phase-A pool and a
    phase-C pool so G6's 64KB/part weight set fits; G5 runs half-width
    j-blocks (JB=256) so its weight ring is 2x8KB.
  - ACT dummy square outputs all land in one junk tile (FIFO-serial
    engine, WAW is free); dot-sums get a 2-buf ring.
"""

import threading
from contextlib import ExitStack

import ml_dtypes
import numpy as np

import concourse.bacc as bacc
import concourse.mybir as mybir
import concourse.tile as tile
from concourse.bass_utils import run_bass_kernel_spmd
from concourse.masks import make_identity

F32 = mybir.dt.float32
F16 = mybir.dt.float16
F8 = mybir.dt.float8e4
AF = mybir.ActivationFunctionType
OP = mybir.AluOpType
PM = mybir.MatmulPerfMode

N_CORES = 8
B, D = 4096, 2048
BL = B // N_CORES          # 512 rows per core
P = 128                    # partitions
NB = BL // P               # 4 batch tiles per core
KC = D // P                # 16 contraction chunks
JB = 512                   # GEMM j-block width (fp8 + G6)
NJ = D // JB               # 4 j-blocks
JB5 = 256                  # G5 j-block width
NJ5 = D // JB5             # 8 j-blocks

SA = 2048.0                # fp8 activation scale
SW = 1024.0                # fp8 weight scale
RS8 = SA * SW

EPS = 1e-5

T_COEF = [1.0, -0.333333329, 0.133333183, -0.05396619044, 0.02185495027,
          -0.008803496923, 0.003439097315, -0.001203079678, 0.0003169332143,
          -4.391237846e-05]
G_COEF = [0.9986986426, 0.3781016454, -0.4412363167, 5.173515875, -23.8123998,
          72.00036756, -137.3859594, 162.2957814, -108.0106185, 31.34506744]


def _build():
    nc = bacc.Bacc(None, target_bir_lowering=False, debug=False)

    hx16_d = nc.dram_tensor("hx16", [P, NB, D], F16, kind="ExternalInput")
    x16_d = nc.dram_tensor("x16", [P, NB, D], F16, kind="ExternalInput")
    xT8_d = nc.dram_tensor("xT8", [P, KC, BL], F8, kind="ExternalInput")
    hxT8_d = nc.dram_tensor("hxT8", [P, KC, BL], F8, kind="ExternalInput")
    xT16_d = nc.dram_tensor("xT16", [P, KC, BL], F16, kind="ExternalInput")
    w8_d = {
        name: nc.dram_tensor(name, [NJ, P, KC, JB], F8, kind="ExternalInput")
        for name in ["wr8", "wz8", "ur8", "uz8"]
    }
    uw16_d = nc.dram_tensor("uw16", [NJ5, P, KC, JB5], F16,
                            kind="ExternalInput")
    ww16_d = nc.dram_tensor("ww16", [NJ, P, KC, JB], F16,
                            kind="ExternalInput")
    brzw_d = nc.dram_tensor("brzw16", [1, 3 * D], F16, kind="ExternalInput")
    out_d = nc.dram_tensor("out", [BL, D], F32, kind="ExternalOutput")

    with ExitStack() as ctx:
        tc = ctx.enter_context(tile.TileContext(nc))
        dve, sca, pe = nc.vector, nc.scalar, nc.tensor

        # ---------------- persistent pools (left side) ----------------
        scal = ctx.enter_context(tc.tile_pool(name="scal", bufs=1))
        cst = ctx.enter_context(tc.tile_pool(name="cst", bufs=1))
        slot = ctx.enter_context(tc.tile_pool(name="slot", bufs=1))  # B,C,D
        hxp = ctx.enter_context(tc.tile_pool(name="hxp", bufs=1))
        jkp = ctx.enter_context(tc.tile_pool(name="jkp", bufs=1))
        sump = ctx.enter_context(tc.tile_pool(name="sump", bufs=2))
        bwp = ctx.enter_context(tc.tile_pool(name="bwp", bufs=1))
        # right side: pT persists; weight pools stack above it
        ptp = ctx.enter_context(tc.tile_pool(name="ptp", bufs=1, side="right"))
        pmm = ctx.enter_context(tc.tile_pool(name="pmm", bufs=6, space="PSUM"))
        ptr = ctx.enter_context(tc.tile_pool(name="ptr", bufs=2, space="PSUM"))

        def stile(w=NB, name="s", pt=False):
            if pt:
                return scal.tile([P, w], F32, tag=f"P_{name}", bufs=1,
                                 name=name)
            return scal.tile([P, w], F32, tag=f"scal{w}",
                             bufs=(32 if w <= NB else 16), name=name)

        def slot_group(pool, nm):
            t = pool.tile([P, NB, D], F16, tag=nm, name=nm)
            return t, [t[:, bt, :] for bt in range(NB)]

        B_big, Bs = slot_group(slot, "B")
        C_big, C = slot_group(slot, "C")
        D_big, Ds = slot_group(slot, "D")
        hx_t = hxp.tile([P, NB, D], F16, tag="hx", name="hx")
        hx = [hx_t[:, bt, :] for bt in range(NB)]
        pT = ptp.tile([P, KC, BL], F16, tag="pT", name="pT")
        junk_act = jkp.tile([P, D], F16, tag="jact", name="jact")
        junk_dve = jkp.tile([P, D], F16, tag="jdve", name="jdve")
        bw_bc = bwp.tile([P, D], F16, tag="bw_bc", name="bw_bc")
        # reserve the sum-ring's stack slot before any scoped pool
        sump.tile([P, D], F16, tag="ss", bufs=2, name="ss_pre")

        ident16 = cst.tile([P, P], F16, tag="id16", name="id16")

        # ---------------- tiny-op helpers ----------------
        def sq_accum_act(t, acc_col):
            sca.activation(junk_act, t, AF.Square, accum_out=acc_col)

        def dot_accum(a, b, acc_col):
            # DVE stt dot (1x mode, 2.26us full-width)
            dve.scalar_tensor_tensor(junk_dve, a, 1.0, b, OP.mult, OP.mult,
                                     accum_out=acc_col)

        def sum_sq(a, b, acc_col):
            # s = a+b on DVE, |s|^2 on ACT -> acc_col
            s = sump.tile([P, D], F16, tag="ss", bufs=2, name="ss")
            dve.tensor_tensor(s, a, b, OP.add)
            sca.activation(junk_act, s, AF.Square, accum_out=acc_col)

        def dot_fin(dcols, n2s_cols, n2a_cols, n2b_cols):
            # d = 0.5*(|a+b|^2 - |a|^2 - |b|^2), width-4 narrow finalize
            dve.tensor_tensor(dcols, n2s_cols, n2a_cols, OP.subtract)
            dve.tensor_tensor(dcols, dcols, n2b_cols, OP.subtract)
            dve.tensor_scalar(dcols, dcols, 0.5, None, OP.mult)

        def poly(dst, u, coefs):
            dve.tensor_scalar(dst, u, float(coefs[-1]), float(coefs[-2]),
                              OP.mult, OP.add)
            for ck in reversed(coefs[:-2]):
                dve.tensor_tensor(dst, dst, u, OP.mult)
                dve.tensor_scalar(dst, dst, float(ck), None, OP.add)

        def t_poly(u, w=NB, pt=False, name="tp"):
            uc = stile(w, "uc")
            dve.tensor_scalar(uc, u, 0.95, None, OP.min)
            o = stile(w, name, pt=pt)
            poly(o, uc, T_COEF)
            return o

        def g_poly(u, w=NB, pt=False, name="gp"):
            uc = stile(w, "ug")
            dve.tensor_scalar(uc, u, 0.10, 0.80, OP.max, OP.min)
            o = stile(w, name, pt=pt)
            poly(o, uc, G_COEF)
            return o

        def mobius(al_a, x2, al_b, y2, dab, n2a, n2b, neg_a=False, pfx="m"):
            """Returns ua, ub, n2m = |ua*a + ub*b|^2 (width from dab)."""
            w = dab.shape[-1]
            xy = stile(w, name="xy")
            dve.tensor_tensor(xy, al_a, al_b, OP.mult)
            dve.tensor_tensor(xy, xy, dab, OP.mult)
            if neg_a:
                dve.tensor_scalar(xy, xy, -1.0, None, OP.mult)
            txy1 = stile(w, name="txy1")
            dve.tensor_scalar(txy1, xy, 2.0, 1.0, OP.mult, OP.add)
            numa = stile(w, name="numa")
            dve.tensor_tensor(numa, txy1, y2, OP.add)
            den = stile(w, name="den")
            dve.tensor_tensor(den, x2, y2, OP.mult)
            dve.tensor_tensor(den, den, txy1, OP.add)
            dve.tensor_scalar(den, den, float(EPS), None, OP.max)
            rden = stile(w, name="rden")
            dve.reciprocal(rden, den)
            ua = stile(w, name=f"{pfx}_ua", pt=True)
            dve.tensor_tensor(ua, numa, al_a, OP.mult)
            dve.tensor_tensor(ua, ua, rden, OP.mult)
            if neg_a:
                dve.tensor_scalar(ua, ua, -1.0, None, OP.mult)
            ub = stile(w, name=f"{pfx}_ub", pt=True)
            dve.tensor_scalar(ub, x2, -1.0, 1.0, OP.mult, OP.add)
            dve.tensor_tensor(ub, ub, al_b, OP.mult)
            dve.tensor_tensor(ub, ub, rden, OP.mult)
            t1 = stile(w, name="t1")
            dve.tensor_tensor(t1, ua, ua, OP.mult)
            dve.tensor_tensor(t1, t1, n2a, OP.mult)
            t2 = stile(w, name="t2")
            dve.tensor_tensor(t2, ub, ub, OP.mult)
            dve.tensor_tensor(t2, t2, n2b, OP.mult)
            t3 = stile(w, name="t3")
            dve.tensor_tensor(t3, ua, ub, OP.mult)
            dve.tensor_tensor(t3, t3, dab, OP.mult)
            n2m = stile(w, name=f"{pfx}_n2m", pt=True)
            dve.scalar_tensor_tensor(n2m, t3, 2.0, t1, OP.mult, OP.add)
            dve.tensor_tensor(n2m, n2m, t2, OP.add)
            return ua, ub, n2m

        def branch(n2ab, dab, pfx):
            """n2ab: [P, 2w] (|a|^2 cols then |b|^2 cols) -> ua, ub, beta."""
            w = dab.shape[-1]
            al = t_poly(n2ab, w=2 * w)
            x2y2 = stile(2 * w, "x2y2")
            dve.tensor_tensor(x2y2, al, al, OP.mult)
            dve.tensor_tensor(x2y2, x2y2, n2ab, OP.mult)
            ua, ub, n2m = mobius(al[:, 0:w], x2y2[:, 0:w],
                                 al[:, w:], x2y2[:, w:],
                                 dab, n2ab[:, 0:w], n2ab[:, w:], pfx=pfx)
            return ua, ub, g_poly(n2m, w=w, pt=True, name=f"{pfx}_beta")

        def combine(dst, a, ua_col, b, ub_col):
            """dst = ua*a + ub*b via ts+ts+add (a, b scaled in place)."""
            dve.tensor_scalar(a, a, ua_col, None, OP.mult)
            dve.tensor_scalar(b, b, ub_col, None, OP.mult)
            dve.tensor_tensor(dst, a, b, OP.add)

        # persistent scalar columns
        n2xh = stile(2 * NB, "n2xh", pt=True)   # cols 0-3 |x|^2, 4-7 |hx|^2
        ones = stile(name="ones", pt=True)
        n2ab_r = stile(2 * NB, "n2ab_r", pt=True)
        n2ab_z = stile(2 * NB, "n2ab_z", pt=True)
        n2w = stile(2 * NB, "n2w", pt=True)   # [v6_0 v6_1 v5_0 v5_1 | ...]
        n2s12 = stile(name="n2s12", pt=True)
        n2s34 = stile(name="n2s34", pt=True)
        n2szh = stile(name="n2szh", pt=True)
        n2shq = stile(name="n2shq", pt=True)
        d12 = stile(name="d12", pt=True)
        d34 = stile(name="d34", pt=True)
        d65 = stile(name="d65", pt=True)
        n2zh = stile(name="n2zh", pt=True)
        dhzh = stile(name="dhzh", pt=True)
        n2q = stile(name="n2q", pt=True)
        dhq = stile(name="dhq", pt=True)
        n2zq = stile(name="n2zq", pt=True)
        dzhzq = stile(name="dzhzq", pt=True)
        dhzq = stile(name="dhzq", pt=True)
        uo_a = stile(name="uo_a", pt=True)
        czh_a = stile(name="czh_a", pt=True)
        czq_a = stile(name="czq_a", pt=True)
        wwslabs = [None] * NJ

        # ---------------- GEMM machinery ----------------
        def gemm_fp8(wpool, wt_dram, actT, scale, v_dst,
                     bias_bc=None, bias_off=0, inject=None, pre=None):
            """fp8 DoubleRow GEMM. bias_bc None -> ACT copies (scale col);
            else DVE stt copies (psum*scale + bias)."""
            for js in range(NJ):
                if pre is not None and js == 0:
                    slab = pre
                else:
                    slab = wpool.tile([P, KC, JB], F8, tag="w8", bufs=2,
                                      name="w8")
                    nc.sync.dma_start(out=slab, in_=wt_dram[js, :, :, :])
                if inject is not None and js in inject:
                    inject[js]()
                for bt in range(NB):
                    ps = pmm.tile([P, JB], F32, tag="mm", name="mm")
                    for c in range(KC // 2):
                        pe.matmul(
                            ps,
                            actT[:, 2 * c:2 * c + 2, bt * P:(bt + 1) * P],
                            slab[:, 2 * c:2 * c + 2, :],
                            start=(c == 0),
                            stop=(c == KC // 2 - 1),
                            perf_mode=PM.DoubleRow,
                        )
                    dst = v_dst[bt][:, js * JB:(js + 1) * JB]
                    if bias_bc is None:
                        sca.activation(out=dst, in_=ps, func=AF.Copy,
                                       scale=scale[:, bt:bt + 1])
                    else:
                        off = bias_off + js * JB
                        dve.scalar_tensor_tensor(
                            dst, ps, scale[:, bt:bt + 1],
                            bias_bc[:, off:off + JB], OP.mult, OP.add)

        def gemm_g5(wpool, actT, scale, v_dst, bias_bc, inject=None,
                    pre=None):
            """fp16 GEMM, half-width j-blocks (JB5), DVE stt copies."""
            for js in range(NJ5):
                if pre is not None and js == 0:
                    slab = pre
                else:
                    slab = wpool.tile([P, KC, JB5], F16, tag="w16h", bufs=2,
                                      name="w16h")
                    nc.sync.dma_start(out=slab, in_=uw16_d[js, :, :, :])
                if inject is not None and js in inject:
                    inject[js]()
                for bt in range(NB):
                    ps = pmm.tile([P, JB], F32, tag="mm", name="mm")
                    for c in range(KC):
                        pe.matmul(
                            ps[:, 0:JB5],
                            actT[:, c, bt * P:(bt + 1) * P],
                            slab[:, c, :],
                            start=(c == 0),
                            stop=(c == KC - 1),
                        )
                    off = js * JB5
                    dve.scalar_tensor_tensor(
                        v_dst[bt][:, off:off + JB5], ps[:, 0:JB5],
                        scale[:, bt:bt + 1], bias_bc[:, off:off + JB5],
                        OP.mult, OP.add)

        # =============== pre-created right/left stacked tiles ===============
        # (tile creation fixes pool stack order; DMAs are issued later)
        xp16 = ctx.enter_context(ExitStack())
        xT16 = xp16.enter_context(tc.tile_pool(name="xp16", bufs=1)).tile(
            [P, KC, BL], F16, tag="aT_xT16", name="xT16")
        wp16s = xp16.enter_context(tc.tile_pool(name="wp16", bufs=1))
        uw_s0 = wp16s.tile([P, KC, JB5], F16, tag="w16h", bufs=2, name="w16h")
        brzp = ExitStack()
        brz_bc = brzp.enter_context(tc.tile_pool(name="brzp", bufs=1)).tile(
            [P, 2 * D], F16, tag="brz_bc", name="brz_bc")
        slotAp = ExitStack()
        A_big, A = slot_group(
            slotAp.enter_context(tc.tile_pool(name="slotA1", bufs=1)), "A1")

        with ExitStack() as phaseA:
            early = phaseA.enter_context(tc.tile_pool(name="early", bufs=1))
            w8p = phaseA.enter_context(tc.tile_pool(name="w8p", bufs=1))
            xp8 = phaseA.enter_context(tc.tile_pool(name="xp8", bufs=1))

            make_identity(nc, ident16)

            # DMA priority: hxT8, hx16, wr8-s0, x16, then the rest
            hxT8 = early.tile([P, KC, BL], F8, tag="aT_hxT8", name="hxT8")
            nc.sync.dma_start(out=hxT8, in_=hxT8_d[:, :, :])
            nc.sync.dma_start(out=hx_t, in_=hx16_d[:, :, :])
            wr_s0 = w8p.tile([P, KC, JB], F8, tag="w8", bufs=2, name="w8")
            nc.sync.dma_start(out=wr_s0, in_=w8_d["wr8"][0, :, :, :])
            nc.sync.dma_start(out=D_big, in_=x16_d[:, :, :])

            # warm the sigmoid table set (has tanh/square/copy too)
            dve.memset(ones, 1.0)
            warm = stile(name="warm", pt=True)
            sca.activation(warm, ones, AF.Sigmoid)

            # |hx|^2, |x|^2 (ACT) -> s_h, s_x
            for bt in range(NB):
                sq_accum_act(hx[bt], n2xh[:, NB + bt:NB + bt + 1])
            for bt in range(NB):
                sq_accum_act(Ds[bt], n2xh[:, bt:bt + 1])
            s_h = g_poly(n2xh[:, NB:], pt=True, name="s_h")
            sc8_h = stile(name="sc8_h", pt=True)
            dve.tensor_scalar(sc8_h, s_h, 1.0 / RS8, None, OP.mult)
            s_x = g_poly(n2xh[:, 0:NB], pt=True, name="s_x")
            sc8_x = stile(name="sc8_x", pt=True)
            dve.tensor_scalar(sc8_x, s_x, 1.0 / RS8, None, OP.mult)

            def a_loads():
                xT8l = xp8.tile([P, KC, BL], F8, tag="aT_xT8", name="xT8")
                nc.sync.dma_start(out=xT8l, in_=xT8_d[:, :, :])
                nc.sync.dma_start(
                    out=brz_bc,
                    in_=brzw_d[0:1, 0:2 * D].partition_broadcast(P))
                nc.sync.dma_start(
                    out=bw_bc,
                    in_=brzw_d[0:1, 2 * D:3 * D].partition_broadcast(P))
                return xT8l

            xT8 = [None]

            # ---- G1: v1 = s_h*(hx @ w_r.T) -> A (ACT copies) ----
            gemm_fp8(w8p, w8_d["wr8"], hxT8, sc8_h, A, pre=wr_s0,
                     inject={1: lambda: xT8.__setitem__(0, a_loads())})
            for bt in range(NB):
                sq_accum_act(A[bt], n2ab_r[:, bt:bt + 1])

            # ---- G2: v2 = s_x*(x @ u_r.T) + br -> C (DVE stt copies) ----
            gemm_fp8(w8p, w8_d["ur8"], xT8[0], sc8_x, C,
                     bias_bc=brz_bc, bias_off=0)
            for bt in range(NB):
                sq_accum_act(C[bt], n2ab_r[:, NB + bt:NB + bt + 1])
                sum_sq(A[bt], C[bt], n2s12[:, bt:bt + 1])
            dot_fin(d12, n2s12, n2ab_r[:, 0:NB], n2ab_r[:, NB:])

            # ---- r branch -> m1 -> r -> p (overlaps G3) ----
            ua1, ub1, b1 = branch(n2ab_r, d12, "r")
            for bt in range(NB):
                combine(A[bt], A[bt], ua1[:, bt:bt + 1], C[bt],
                        ub1[:, bt:bt + 1])

            # ---- G3: v3 = s_h*(hx @ w_z.T) -> B (ACT copies) ----
            def g3_post():
                for bt in range(NB):
                    sca.activation(A[bt], A[bt], AF.Sigmoid,
                                   scale=b1[:, bt:bt + 1])      # r in place
                    dve.tensor_tensor(A[bt], A[bt], hx[bt], OP.mult)  # p

            gemm_fp8(w8p, w8_d["wz8"], hxT8, sc8_h, Bs,
                     inject={2: g3_post})
            for bt in range(NB):
                sq_accum_act(Bs[bt], n2ab_z[:, bt:bt + 1])
            # xT16 for G5 (tile pre-created on left)
            nc.sync.dma_start(out=xT16, in_=xT16_d[:, :, :])

            # ---- G4: v4 = s_x*(x @ u_z.T) + bz -> D (DVE stt copies) ----
            def g4_post():
                # prefetch G5's first half-slab
                nc.sync.dma_start(out=uw_s0, in_=uw16_d[0, :, :, :])

            gemm_fp8(w8p, w8_d["uz8"], xT8[0], sc8_x, Ds,
                     bias_bc=brz_bc, bias_off=D, inject={3: g4_post})
            for bt in range(NB):
                sq_accum_act(Ds[bt], n2ab_z[:, NB + bt:NB + bt + 1])
                sum_sq(Bs[bt], Ds[bt], n2s34[:, bt:bt + 1])
            dot_fin(d34, n2s34, n2ab_z[:, 0:NB], n2ab_z[:, NB:])

            # ---- z branch -> m2 -> z ----
            ua2, ub2, b2 = branch(n2ab_z, d34, "z")
            for bt in range(NB):
                combine(Bs[bt], Bs[bt], ua2[:, bt:bt + 1], Ds[bt],
                        ub2[:, bt:bt + 1])
                sca.activation(Ds[bt], Bs[bt], AF.Sigmoid,
                               scale=b2[:, bt:bt + 1])       # z -> D

            # ---- transpose p (A) -> pT on PE ----
            for bt in range(NB):
                for cp in range(KC // 4):
                    ps = ptr.tile([P, JB], F16, tag="tr", name="tr")
                    for k in range(4):
                        pe.transpose(
                            ps[:, k * P:(k + 1) * P],
                            A[bt][:, (cp * 4 + k) * P:(cp * 4 + k + 1) * P],
                            ident16,
                        )
                    dve.tensor_copy(
                        out=pT[:, cp * 4:cp * 4 + 4, bt * P:(bt + 1) * P],
                        in_=ps.rearrange("p (c b) -> p c b", c=4),
                    )
        slotAp.close()   # A free until phase C
        brzp.close()     # r/z biases dead after G4

        # =============== phase B: G5 + zh work + ww prefetch ===============
        wwpC = ctx.enter_context(ExitStack())
        wwpool = wwpC.enter_context(
            tc.tile_pool(name="wwp", bufs=1, side="right"))

        def prefetch_ww(js):
            def go():
                wwslabs[js] = wwpool.tile([P, KC, JB], F16, tag=f"ww{js}",
                                          name=f"ww{js}")
                nc.sync.dma_start(out=wwslabs[js],
                                  in_=ww16_d[js, :, :, :])
            return go

        def zh_work():
            # zh = z*hx -> B; |zh|^2 (ACT); <hx,zh> via sum (dhzh)
            for bt in range(NB):
                dve.tensor_tensor(Bs[bt], Ds[bt], hx[bt], OP.mult)
                sq_accum_act(Bs[bt], n2zh[:, bt:bt + 1])
                sum_sq(hx[bt], Bs[bt], n2szh[:, bt:bt + 1])
            dot_fin(dhzh, n2szh, n2xh[:, NB:], n2zh)

        # G5: v5 = s_x*(x @ u_w.T) + bw -> C (DVE stt copies)
        gemm_g5(wp16s, xT16, s_x, C, bw_bc, pre=uw_s0,
                inject={0: prefetch_ww(0), 1: prefetch_ww(1),
                        2: zh_work, 3: prefetch_ww(2), 5: prefetch_ww(3)})
        # |v5|^2 -> n2w pair layout cols (ACT)
        for bt in range(NB):
            vcol = (bt // 2) * 4 + 2 + (bt % 2)
            sq_accum_act(C[bt], n2w[:, vcol:vcol + 1])
        xp16.close()   # frees xT16 + G5 weight ring

        # =============== phase C: G6 bt-major + tail ===============
        with ExitStack() as phaseC:
            outp = phaseC.enter_context(
                tc.tile_pool(name="outp", bufs=1, side="right"))
            outt = [outp.tile([P, D], F32, tag=f"o{i}", name=f"o{i}")
                    for i in range(2)]
            A_big, A = slot_group(
                phaseC.enter_context(tc.tile_pool(name="slotA2", bufs=1)),
                "A2")

            b3s = [None, None]

            def pair_chain(g):
                """branch3 + m3 for bts (2g, 2g+1); m3 scale-halves on ACT."""
                sl = slice(4 * g, 4 * g + 4)
                ua3, ub3, b3 = branch(n2w[:, sl], d65[:, 2 * g:2 * g + 2],
                                      f"w{g}")
                for i, bt in enumerate((2 * g, 2 * g + 1)):
                    sca.activation(A[bt], A[bt], AF.Copy,
                                   scale=ua3[:, i:i + 1])
                    sca.activation(C[bt], C[bt], AF.Copy,
                                   scale=ub3[:, i:i + 1])
                    dve.tensor_tensor(A[bt], A[bt], C[bt], OP.add)  # m3
                b3s[g] = b3

            def pair_tanh(g):
                for i, bt in enumerate((2 * g, 2 * g + 1)):
                    sca.activation(C[bt], A[bt], AF.Tanh,
                                   scale=b3s[g][:, i:i + 1])     # q -> C

            def pair_postq(g):
                for bt in (2 * g, 2 * g + 1):
                    # dhq via sum identity: s = hx + q -> A (m3 dead)
                    dve.tensor_tensor(A[bt], hx[bt], C[bt], OP.add)
                    sca.activation(junk_act, A[bt], AF.Square,
                                   accum_out=n2shq[:, bt:bt + 1])
                    sq_accum_act(C[bt], n2q[:, bt:bt + 1])
                    # zq = z*q -> D (z dies)
                    dve.tensor_tensor(Ds[bt], Ds[bt], C[bt], OP.mult)
                    sq_accum_act(Ds[bt], n2zq[:, bt:bt + 1])
                    # dhzq = <hx,zq> = <zh,q>; dzhzq = <zh,zq> (DVE stt)
                    dot_accum(Bs[bt], C[bt], dhzq[:, bt:bt + 1])
                    dot_accum(Bs[bt], Ds[bt], dzhzq[:, bt:bt + 1])

            def tail_scalars(g):
                """d/e mobius scalar chain for pair g -> uo, c_zh, c_zq."""
                sl = slice(2 * g, 2 * g + 2)
                n2h = n2xh[:, NB + 2 * g:NB + 2 * g + 2]
                dq = dhq[:, sl]
                dve.tensor_tensor(dq, n2shq[:, sl], n2q[:, sl], OP.subtract)
                dve.tensor_tensor(dq, dq, n2h, OP.subtract)
                dve.tensor_scalar(dq, dq, 0.5, None, OP.mult)

                delta = t_poly(n2q[:, sl], w=2, name=f"delta{g}")
                thq2 = stile(2, name="thq2")
                dve.tensor_tensor(thq2, delta, delta, OP.mult)
                dve.tensor_tensor(thq2, thq2, n2q[:, sl], OP.mult)
                ua_d, ub_d, n2d = mobius(ones[:, sl], n2h, delta, thq2,
                                         dq, n2h, n2q[:, sl],
                                         neg_a=True, pfx=f"d{g}")
                beta_d = g_poly(n2d, w=2, name=f"beta_d{g}")

                n2e = stile(2, name="n2e")
                t1 = stile(2, name="te1")
                dve.tensor_tensor(t1, ua_d, ua_d, OP.mult)
                dve.tensor_tensor(t1, t1, n2zh[:, sl], OP.mult)
                t2 = stile(2, name="te2")
                dve.tensor_tensor(t2, ub_d, ub_d, OP.mult)
                dve.tensor_tensor(t2, t2, n2zq[:, sl], OP.mult)
                t3 = stile(2, name="te3")
                dve.tensor_tensor(t3, ua_d, ub_d, OP.mult)
                dve.tensor_tensor(t3, t3, dzhzq[:, sl], OP.mult)
                dve.scalar_tensor_tensor(n2e, t3, 2.0, t1, OP.mult, OP.add)
                dve.tensor_tensor(n2e, n2e, t2, OP.add)

                nt2 = stile(2, name="nt2")
                dve.tensor_tensor(nt2, beta_d, beta_d, OP.mult)
                dve.tensor_tensor(nt2, nt2, n2e, OP.mult)
                tt = t_poly(nt2, w=2, name=f"tt{g}")
                eps_s = stile(2, name="eps_s")
                dve.tensor_tensor(eps_s, tt, beta_d, OP.mult)
                tht2 = stile(2, name="tht2")
                dve.tensor_tensor(tht2, tt, tt, OP.mult)
                dve.tensor_tensor(tht2, tht2, nt2, OP.mult)
                dhe = stile(2, name="dhe")
                dve.tensor_tensor(dhe, ua_d, dhzh[:, sl], OP.mult)
                t4 = stile(2, name="te4")
                dve.tensor_tensor(t4, ub_d, dhzq[:, sl], OP.mult)
                dve.tensor_tensor(dhe, dhe, t4, OP.add)

                uo, vo, _ = mobius(ones[:, sl], n2h, eps_s, tht2, dhe,
                                   n2h, n2e, pfx=f"o{g}")
                dve.tensor_copy(out=uo_a[:, sl], in_=uo)
                dve.tensor_tensor(czh_a[:, sl], vo, ua_d, OP.mult)
                dve.tensor_tensor(czq_a[:, sl], vo, ub_d, OP.mult)

            def final_bt(bt):
                dve.tensor_scalar(Ds[bt], Ds[bt], czq_a[:, bt:bt + 1], None,
                                  OP.mult)
                dve.scalar_tensor_tensor(Ds[bt], Bs[bt], czh_a[:, bt:bt + 1],
                                         Ds[bt], OP.mult, OP.add)
                o = outt[bt % 2]
                dve.scalar_tensor_tensor(o, hx[bt], uo_a[:, bt:bt + 1],
                                         Ds[bt], OP.mult, OP.add)
                nc.sync.dma_start(out=out_d[bt * P:(bt + 1) * P, :], in_=o)

            # ---- G6 bt-major: v6 = s_h*(p @ w.T) -> A (ACT copies) ----
            for bt in range(NB):
                for js in range(NJ):
                    ps = pmm.tile([P, JB], F32, tag="mm", name="mm")
                    for c in range(KC):
                        pe.matmul(
                            ps,
                            pT[:, c, bt * P:(bt + 1) * P],
                            wwslabs[js][:, c, :],
                            start=(c == 0),
                            stop=(c == KC - 1),
                        )
                    sca.activation(
                        out=A[bt][:, js * JB:(js + 1) * JB], in_=ps,
                        func=AF.Copy, scale=s_h[:, bt:bt + 1],
                    )
                col = (bt // 2) * 4 + (bt % 2)
                sq_accum_act(A[bt], n2w[:, col:col + 1])   # ACT: |v6|^2
                if bt == 3:
                    # pair0 epilogue: ready DVE work ahead of d65-bt3's wait
                    tail_scalars(0)
                    final_bt(0)
                    final_bt(1)
                dcol = (bt // 2) * 2 + (bt % 2)
                dot_accum(A[bt], C[bt], d65[:, dcol:dcol + 1])  # DVE stt
                if bt == 1:
                    pair_chain(0)
                if bt == 2:
                    pair_tanh(0)
                    pair_postq(0)
            pair_chain(1)
            pair_tanh(1)
            pair_postq(1)
            tail_scalars(1)
            final_bt(2)
            final_bt(3)

    nc.compile()
    return nc


_BUILD_LOCK = threading.Lock()
_NC_CACHE = {}


def _get_nc():
    with _BUILD_LOCK:
        if "nc" not in _NC_CACHE:
            _NC_CACHE["nc"] = _build()
        return _NC_CACHE["nc"]


def _prep_in_maps(inputs):
    f8 = ml_dtypes.float8_e4m3
    x = np.asarray(inputs["x"], dtype=np.float32)
    hx = np.asarray(inputs["hx"], dtype=np.float32)

    def swz_w(a, scale, dt, jb, nj):
        # W [D_out, D_in] -> wT [K=D_in, J=D_out] -> [nj, P, KC, jb] js-major
        wt = np.asarray(a, np.float32).T * scale
        if dt is f8:
            wt = np.clip(wt, -240, 240)
        return np.ascontiguousarray(
            wt.reshape(KC, P, nj, jb).transpose(2, 1, 0, 3)
        ).astype(dt)

    def swz_aT(aT, scale, dt):
        a = aT * scale
        if dt is f8:
            a = np.clip(a, -240, 240)
        return np.ascontiguousarray(
            a.reshape(KC, P, BL).transpose(1, 0, 2)
        ).astype(dt)

    def swz_nat(a):
        return np.ascontiguousarray(
            np.asarray(a, np.float32).reshape(NB, P, D).transpose(1, 0, 2)
        ).astype(np.float16)

    weights = {
        "wr8": swz_w(inputs["w_r"], SW, f8, JB, NJ),
        "ur8": swz_w(inputs["u_r_w"], SW, f8, JB, NJ),
        "wz8": swz_w(inputs["w_z"], SW, f8, JB, NJ),
        "uz8": swz_w(inputs["u_z_w"], SW, f8, JB, NJ),
        "uw16": swz_w(inputs["u_w"], 1.0, np.float16, JB5, NJ5),
        "ww16": swz_w(inputs["w"], 1.0, np.float16, JB, NJ),
    }
    brzw = np.concatenate([
        np.asarray(inputs["u_r_b"], np.float32),
        np.asarray(inputs["u_z_b"], np.float32),
        np.asarray(inputs["u_b"], np.float32),
    ]).reshape(1, 3 * D)
    biases = {"brzw16": brzw.astype(np.float16)}

    in_maps = []
    for c in range(N_CORES):
        xs = x[c * BL:(c + 1) * BL]
        hs = hx[c * BL:(c + 1) * BL]
        xsT = np.ascontiguousarray(xs.T)
        hsT = np.ascontiguousarray(hs.T)
        m = {
            "x16": swz_nat(xs),
            "hx16": swz_nat(hs),
            "xT8": swz_aT(xsT, SA, f8),
            "hxT8": swz_aT(hsT, SA, f8),
            "xT16": swz_aT(xsT, 1.0, np.float16),
        }
        m.update(weights)
        m.update(biases)
        in_maps.append(m)
    return in_maps


def kernel(**inputs: np.ndarray) -> np.ndarray:
    in_maps = _prep_in_maps(inputs)
    nc = _get_nc()
    res = run_bass_kernel_spmd(nc, in_maps, core_ids=list(range(N_CORES)))
    return np.concatenate([r["out"] for r in res.results], axis=0)


# revision 32
# speedup vs baseline: 1.0976x; 1.0366x over previous
"""Trainium2 Bass kernel for the hyperbolic (Poincare-ball) GRU cell.

Data-parallel over batch across 8 NeuronCores, no collectives.

v3 schedule (vs 392us baseline / 380us v2):
  - GEMMs G1-G4 fp8 DoubleRow, G5/G6 fp16 (fp8 fails the 2e-2 gate).
    G6 bt-major with all 4 weight slabs resident so each batch-tile's
    tail chain starts as soon as its v6 lands.
  - Rank-1 bias matmuls removed: biases live as partition-broadcast SBUF
    tiles, added by the DVE copy (stt: psum*scale + bias).
  - Engine balance (measured costs: DVE stt 2.26us full-width ALWAYS, TT
    1.14, TS 0.6, ACT pass 1.93): squares on ACT; dots d12/d34/dhzh/dhq
    via |a+b|^2 sum-identity (DVE add + ACT square), d65/dzhzq/dhzq as
    DVE stt; m1/m2 as ts+ts+add on DVE, m3 scale-halves on ACT.
  - SBUF: left/right pool stacks; A-slot split into a phase-A pool and a
    phase-C pool so G6's 64KB/part weight set fits; G5 runs half-width
    j-blocks (JB=256) so its weight ring is 2x8KB.
  - ACT dummy square outputs all land in one junk tile (FIFO-serial
    engine, WAW is free); dot-sums get a 2-buf ring.
"""

import threading
from contextlib import ExitStack

import ml_dtypes
import numpy as np

import concourse.bacc as bacc
import concourse.mybir as mybir
import concourse.tile as tile
from concourse.bass_utils import run_bass_kernel_spmd
from concourse.masks import make_identity

F32 = mybir.dt.float32
F16 = mybir.dt.float16
F8 = mybir.dt.float8e4
AF = mybir.ActivationFunctionType
OP = mybir.AluOpType
PM = mybir.MatmulPerfMode

N_CORES = 8
B, D = 4096, 2048
BL = B // N_CORES          # 512 rows per core
P = 128                    # partitions
NB = BL // P               # 4 batch tiles per core
KC = D // P                # 16 contraction chunks
JB = 512                   # GEMM j-block width (fp8 + G6)
NJ = D // JB               # 4 j-blocks
JB5 = 256                  # G5 j-block width
NJ5 = D // JB5             # 8 j-blocks

SA = 2048.0                # fp8 activation scale
SW = 1024.0                # fp8 weight scale
RS8 = SA * SW

EPS = 1e-5

# t(u)=tanh(sqrt u)/sqrt u deg-5 on [0,0.95] (rel err 5.6e-7);
# g(u)=artanh(sqrt u)/sqrt u deg-6 on [0.10,0.80] (rel err 9.7e-5)
T_COEF = [0.9999995687, -0.3332995848, 0.1328995987, -0.05187979588,
          0.01709898883, -0.003227323388]
G_COEF = [1.005682159, 0.2006510111, 1.367833128, -4.873127594,
          11.38167234, -12.7154928, 5.996635306]


def _build():
    nc = bacc.Bacc(None, target_bir_lowering=False, debug=False)

    hx16_d = nc.dram_tensor("hx16", [P, NB, D], F16, kind="ExternalInput")
    x16_d = nc.dram_tensor("x16", [P, NB, D], F16, kind="ExternalInput")
    xT8_d = nc.dram_tensor("xT8", [P, KC, BL], F8, kind="ExternalInput")
    hxT8_d = nc.dram_tensor("hxT8", [P, KC, BL], F8, kind="ExternalInput")
    xT16_d = nc.dram_tensor("xT16", [P, KC, BL], F16, kind="ExternalInput")
    w8_d = {
        name: nc.dram_tensor(name, [NJ, P, KC, JB], F8, kind="ExternalInput")
        for name in ["wr8", "wz8", "ur8", "uz8"]
    }
    uw16_d = nc.dram_tensor("uw16", [NJ5, P, KC, JB5], F16,
                            kind="ExternalInput")
    ww16_d = nc.dram_tensor("ww16", [NJ, P, KC, JB], F16,
                            kind="ExternalInput")
    brzw_d = nc.dram_tensor("brzw16", [1, 3 * D], F16, kind="ExternalInput")
    out_d = nc.dram_tensor("out", [BL, D], F32, kind="ExternalOutput")

    with ExitStack() as ctx:
        tc = ctx.enter_context(tile.TileContext(nc))
        dve, sca, pe = nc.vector, nc.scalar, nc.tensor

        # ---------------- persistent pools (left side) ----------------
        scal = ctx.enter_context(tc.tile_pool(name="scal", bufs=1))
        cst = ctx.enter_context(tc.tile_pool(name="cst", bufs=1))
        slot = ctx.enter_context(tc.tile_pool(name="slot", bufs=1))  # B,C,D
        hxp = ctx.enter_context(tc.tile_pool(name="hxp", bufs=1))
        jkp = ctx.enter_context(tc.tile_pool(name="jkp", bufs=1))
        sump = ctx.enter_context(tc.tile_pool(name="sump", bufs=2))
        bwp = ctx.enter_context(tc.tile_pool(name="bwp", bufs=1))
        # right side: pT persists; weight pools stack above it
        ptp = ctx.enter_context(tc.tile_pool(name="ptp", bufs=1, side="right"))
        pmm = ctx.enter_context(tc.tile_pool(name="pmm", bufs=6, space="PSUM"))
        ptr = ctx.enter_context(tc.tile_pool(name="ptr", bufs=2, space="PSUM"))

        def stile(w=NB, name="s", pt=False):
            if pt:
                return scal.tile([P, w], F32, tag=f"P_{name}", bufs=1,
                                 name=name)
            return scal.tile([P, w], F32, tag=f"scal{w}",
                             bufs=(32 if w <= NB else 16), name=name)

        def slot_group(pool, nm):
            t = pool.tile([P, NB, D], F16, tag=nm, name=nm)
            return t, [t[:, bt, :] for bt in range(NB)]

        B_big, Bs = slot_group(slot, "B")
        C_big, C = slot_group(slot, "C")
        D_big, Ds = slot_group(slot, "D")
        hx_t = hxp.tile([P, NB, D], F16, tag="hx", name="hx")
        hx = [hx_t[:, bt, :] for bt in range(NB)]
        pT = ptp.tile([P, KC, BL], F16, tag="pT", name="pT")
        junk_act = jkp.tile([P, D], F16, tag="jact", name="jact")
        junk_dve = jkp.tile([P, D], F16, tag="jdve", name="jdve")
        bw_bc = bwp.tile([P, D], F16, tag="bw_bc", name="bw_bc")
        # reserve the sum-ring's stack slot before any scoped pool
        sump.tile([P, D], F16, tag="ss", bufs=2, name="ss_pre")

        ident16 = cst.tile([P, P], F16, tag="id16", name="id16")

        # ---------------- tiny-op helpers ----------------
        def sq_accum_act(t, acc_col):
            sca.activation(junk_act, t, AF.Square, accum_out=acc_col)

        def dot_accum(a, b, acc_col):
            # DVE stt dot (1x mode, 2.26us full-width)
            dve.scalar_tensor_tensor(junk_dve, a, 1.0, b, OP.mult, OP.mult,
                                     accum_out=acc_col)

        def sum_sq(a, b, acc_col):
            # s = a+b on DVE, |s|^2 on ACT -> acc_col
            s = sump.tile([P, D], F16, tag="ss", bufs=2, name="ss")
            dve.tensor_tensor(s, a, b, OP.add)
            sca.activation(junk_act, s, AF.Square, accum_out=acc_col)

        def dot_fin(dcols, n2s_cols, n2a_cols, n2b_cols):
            # d = 0.5*(|a+b|^2 - |a|^2 - |b|^2), width-4 narrow finalize
            dve.tensor_tensor(dcols, n2s_cols, n2a_cols, OP.subtract)
            dve.tensor_tensor(dcols, dcols, n2b_cols, OP.subtract)
            dve.tensor_scalar(dcols, dcols, 0.5, None, OP.mult)

        def poly(dst, u, coefs):
            dve.tensor_scalar(dst, u, float(coefs[-1]), float(coefs[-2]),
                              OP.mult, OP.add)
            for ck in reversed(coefs[:-2]):
                dve.tensor_tensor(dst, dst, u, OP.mult)
                dve.tensor_scalar(dst, dst, float(ck), None, OP.add)

        def t_poly(u, w=NB, pt=False, name="tp"):
            uc = stile(w, "uc")
            dve.tensor_scalar(uc, u, 0.95, None, OP.min)
            o = stile(w, name, pt=pt)
            poly(o, uc, T_COEF)
            return o

        def g_poly(u, w=NB, pt=False, name="gp"):
            uc = stile(w, "ug")
            dve.tensor_scalar(uc, u, 0.10, 0.80, OP.max, OP.min)
            o = stile(w, name, pt=pt)
            poly(o, uc, G_COEF)
            return o

        def mobius(al_a, x2, al_b, y2, dab, n2a, n2b, neg_a=False, pfx="m"):
            """Returns ua, ub, n2m = |ua*a + ub*b|^2 (width from dab)."""
            w = dab.shape[-1]
            xy = stile(w, name="xy")
            dve.tensor_tensor(xy, al_a, al_b, OP.mult)
            dve.tensor_tensor(xy, xy, dab, OP.mult)
            if neg_a:
                dve.tensor_scalar(xy, xy, -1.0, None, OP.mult)
            txy1 = stile(w, name="txy1")
            dve.tensor_scalar(txy1, xy, 2.0, 1.0, OP.mult, OP.add)
            numa = stile(w, name="numa")
            dve.tensor_tensor(numa, txy1, y2, OP.add)
            den = stile(w, name="den")
            dve.tensor_tensor(den, x2, y2, OP.mult)
            dve.tensor_tensor(den, den, txy1, OP.add)
            dve.tensor_scalar(den, den, float(EPS), None, OP.max)
            rden = stile(w, name="rden")
            dve.reciprocal(rden, den)
            ua = stile(w, name=f"{pfx}_ua", pt=True)
            dve.tensor_tensor(ua, numa, al_a, OP.mult)
            dve.tensor_tensor(ua, ua, rden, OP.mult)
            if neg_a:
                dve.tensor_scalar(ua, ua, -1.0, None, OP.mult)
            ub = stile(w, name=f"{pfx}_ub", pt=True)
            dve.tensor_scalar(ub, x2, -1.0, 1.0, OP.mult, OP.add)
            dve.tensor_tensor(ub, ub, al_b, OP.mult)
            dve.tensor_tensor(ub, ub, rden, OP.mult)
            t1 = stile(w, name="t1")
            dve.tensor_tensor(t1, ua, ua, OP.mult)
            dve.tensor_tensor(t1, t1, n2a, OP.mult)
            t2 = stile(w, name="t2")
            dve.tensor_tensor(t2, ub, ub, OP.mult)
            dve.tensor_tensor(t2, t2, n2b, OP.mult)
            t3 = stile(w, name="t3")
            dve.tensor_tensor(t3, ua, ub, OP.mult)
            dve.tensor_tensor(t3, t3, dab, OP.mult)
            n2m = stile(w, name=f"{pfx}_n2m", pt=True)
            dve.scalar_tensor_tensor(n2m, t3, 2.0, t1, OP.mult, OP.add)
            dve.tensor_tensor(n2m, n2m, t2, OP.add)
            return ua, ub, n2m

        def branch(n2ab, dab, pfx):
            """n2ab: [P, 2w] (|a|^2 cols then |b|^2 cols) -> ua, ub, beta."""
            w = dab.shape[-1]
            al = t_poly(n2ab, w=2 * w)
            x2y2 = stile(2 * w, "x2y2")
            dve.tensor_tensor(x2y2, al, al, OP.mult)
            dve.tensor_tensor(x2y2, x2y2, n2ab, OP.mult)
            ua, ub, n2m = mobius(al[:, 0:w], x2y2[:, 0:w],
                                 al[:, w:], x2y2[:, w:],
                                 dab, n2ab[:, 0:w], n2ab[:, w:], pfx=pfx)
            return ua, ub, g_poly(n2m, w=w, pt=True, name=f"{pfx}_beta")

        def combine(dst, a, ua_col, b, ub_col):
            """dst = ua*a + ub*b via ts+ts+add (a, b scaled in place)."""
            dve.tensor_scalar(a, a, ua_col, None, OP.mult)
            dve.tensor_scalar(b, b, ub_col, None, OP.mult)
            dve.tensor_tensor(dst, a, b, OP.add)

        # persistent scalar columns
        n2xh = stile(2 * NB, "n2xh", pt=True)   # cols 0-3 |x|^2, 4-7 |hx|^2
        ones = stile(name="ones", pt=True)
        n2ab_r = stile(2 * NB, "n2ab_r", pt=True)
        n2ab_z = stile(2 * NB, "n2ab_z", pt=True)
        n2w = stile(2 * NB, "n2w", pt=True)   # [v6_0 v6_1 v5_0 v5_1 | ...]
        n2s12 = stile(name="n2s12", pt=True)
        n2s34 = stile(name="n2s34", pt=True)
        n2szh = stile(name="n2szh", pt=True)
        n2shq = stile(name="n2shq", pt=True)
        d12 = stile(name="d12", pt=True)
        d34 = stile(name="d34", pt=True)
        d65 = stile(name="d65", pt=True)
        n2zh = stile(name="n2zh", pt=True)
        dhzh = stile(name="dhzh", pt=True)
        n2q = stile(name="n2q", pt=True)
        dhq = stile(name="dhq", pt=True)
        n2zq = stile(name="n2zq", pt=True)
        dzhzq = stile(name="dzhzq", pt=True)
        dhzq = stile(name="dhzq", pt=True)
        uo_a = stile(name="uo_a", pt=True)
        czh_a = stile(name="czh_a", pt=True)
        czq_a = stile(name="czq_a", pt=True)
        wwslabs = [None] * NJ

        # ---------------- GEMM machinery ----------------
        def gemm_fp8(wpool, wt_dram, actT, scale, v_dst,
                     bias_bc=None, bias_off=0, inject=None, pre=None):
            """fp8 DoubleRow GEMM. bias_bc None -> ACT copies (scale col);
            else DVE stt copies (psum*scale + bias)."""
            for js in range(NJ):
                if pre is not None and js == 0:
                    slab = pre
                else:
                    slab = wpool.tile([P, KC, JB], F8, tag="w8", bufs=2,
                                      name="w8")
                    nc.sync.dma_start(out=slab, in_=wt_dram[js, :, :, :])
                if inject is not None and js in inject:
                    inject[js]()
                for bt in range(NB):
                    ps = pmm.tile([P, JB], F32, tag="mm", name="mm")
                    for c in range(KC // 2):
                        pe.matmul(
                            ps,
                            actT[:, 2 * c:2 * c + 2, bt * P:(bt + 1) * P],
                            slab[:, 2 * c:2 * c + 2, :],
                            start=(c == 0),
                            stop=(c == KC // 2 - 1),
                            perf_mode=PM.DoubleRow,
                        )
                    dst = v_dst[bt][:, js * JB:(js + 1) * JB]
                    if bias_bc is None:
                        sca.activation(out=dst, in_=ps, func=AF.Copy,
                                       scale=scale[:, bt:bt + 1])
                    else:
                        off = bias_off + js * JB
                        dve.scalar_tensor_tensor(
                            dst, ps, scale[:, bt:bt + 1],
                            bias_bc[:, off:off + JB], OP.mult, OP.add)

        def gemm_g5(wpool, actT, scale, v_dst, bias_bc, inject=None,
                    pre=None):
            """fp16 GEMM, half-width j-blocks (JB5), DVE stt copies."""
            for js in range(NJ5):
                if pre is not None and js == 0:
                    slab = pre
                else:
                    slab = wpool.tile([P, KC, JB5], F16, tag="w16h", bufs=2,
                                      name="w16h")
                    nc.sync.dma_start(out=slab, in_=uw16_d[js, :, :, :])
                if inject is not None and js in inject:
                    inject[js]()
                for bt in range(NB):
                    ps = pmm.tile([P, JB], F32, tag="mm", name="mm")
                    for c in range(KC):
                        pe.matmul(
                            ps[:, 0:JB5],
                            actT[:, c, bt * P:(bt + 1) * P],
                            slab[:, c, :],
                            start=(c == 0),
                            stop=(c == KC - 1),
                        )
                    off = js * JB5
                    dve.scalar_tensor_tensor(
                        v_dst[bt][:, off:off + JB5], ps[:, 0:JB5],
                        scale[:, bt:bt + 1], bias_bc[:, off:off + JB5],
                        OP.mult, OP.add)

        # =============== pre-created right/left stacked tiles ===============
        # (tile creation fixes pool stack order; DMAs are issued later)
        xp16 = ctx.enter_context(ExitStack())
        xT16 = xp16.enter_context(tc.tile_pool(name="xp16", bufs=1)).tile(
            [P, KC, BL], F16, tag="aT_xT16", name="xT16")
        wp16s = xp16.enter_context(tc.tile_pool(name="wp16", bufs=1))
        uw_s0 = wp16s.tile([P, KC, JB5], F16, tag="w16h", bufs=2, name="w16h")
        brzp = ExitStack()
        brz_bc = brzp.enter_context(tc.tile_pool(name="brzp", bufs=1)).tile(
            [P, 2 * D], F16, tag="brz_bc", name="brz_bc")
        slotAp = ExitStack()
        A_big, A = slot_group(
            slotAp.enter_context(tc.tile_pool(name="slotA1", bufs=1)), "A1")

        with ExitStack() as phaseA:
            early = phaseA.enter_context(tc.tile_pool(name="early", bufs=1))
            w8p = phaseA.enter_context(tc.tile_pool(name="w8p", bufs=1))
            xp8 = phaseA.enter_context(tc.tile_pool(name="xp8", bufs=1))

            make_identity(nc, ident16)

            # DMA priority: hxT8, wr8-s0, hx16, x16, then the rest
            hxT8 = early.tile([P, KC, BL], F8, tag="aT_hxT8", name="hxT8")
            nc.sync.dma_start(out=hxT8, in_=hxT8_d[:, :, :])
            wr_s0 = w8p.tile([P, KC, JB], F8, tag="w8", bufs=2, name="w8")
            nc.sync.dma_start(out=wr_s0, in_=w8_d["wr8"][0, :, :, :])
            nc.sync.dma_start(out=hx_t, in_=hx16_d[:, :, :])
            nc.sync.dma_start(out=D_big, in_=x16_d[:, :, :])

            # warm the sigmoid table set (has tanh/square/copy too)
            dve.memset(ones, 1.0)
            warm = stile(name="warm", pt=True)
            sca.activation(warm, ones, AF.Sigmoid)

            # |hx|^2, |x|^2 (ACT) -> s_h, s_x
            for bt in range(NB):
                sq_accum_act(hx[bt], n2xh[:, NB + bt:NB + bt + 1])
            for bt in range(NB):
                sq_accum_act(Ds[bt], n2xh[:, bt:bt + 1])
            s_h = g_poly(n2xh[:, NB:], pt=True, name="s_h")
            sc8_h = stile(name="sc8_h", pt=True)
            dve.tensor_scalar(sc8_h, s_h, 1.0 / RS8, None, OP.mult)
            s_x = g_poly(n2xh[:, 0:NB], pt=True, name="s_x")
            sc8_x = stile(name="sc8_x", pt=True)
            dve.tensor_scalar(sc8_x, s_x, 1.0 / RS8, None, OP.mult)

            def a_loads():
                xT8l = xp8.tile([P, KC, BL], F8, tag="aT_xT8", name="xT8")
                nc.sync.dma_start(out=xT8l, in_=xT8_d[:, :, :])
                nc.sync.dma_start(
                    out=brz_bc,
                    in_=brzw_d[0:1, 0:2 * D].partition_broadcast(P))
                nc.sync.dma_start(
                    out=bw_bc,
                    in_=brzw_d[0:1, 2 * D:3 * D].partition_broadcast(P))
                return xT8l

            xT8 = [None]

            # ---- G1: v1 = s_h*(hx @ w_r.T) -> A (ACT copies) ----
            gemm_fp8(w8p, w8_d["wr8"], hxT8, sc8_h, A, pre=wr_s0,
                     inject={1: lambda: xT8.__setitem__(0, a_loads())})
            for bt in range(NB):
                sq_accum_act(A[bt], n2ab_r[:, bt:bt + 1])

            # ---- G2: v2 = s_x*(x @ u_r.T) + br -> C (DVE stt copies) ----
            gemm_fp8(w8p, w8_d["ur8"], xT8[0], sc8_x, C,
                     bias_bc=brz_bc, bias_off=0)
            for bt in range(NB):
                sq_accum_act(C[bt], n2ab_r[:, NB + bt:NB + bt + 1])
                sum_sq(A[bt], C[bt], n2s12[:, bt:bt + 1])
            dot_fin(d12, n2s12, n2ab_r[:, 0:NB], n2ab_r[:, NB:])

            # ---- r branch -> m1 -> r -> p (overlaps G3) ----
            ua1, ub1, b1 = branch(n2ab_r, d12, "r")
            for bt in range(NB):
                combine(A[bt], A[bt], ua1[:, bt:bt + 1], C[bt],
                        ub1[:, bt:bt + 1])

            # ---- G3: v3 = s_h*(hx @ w_z.T) -> B (ACT copies) ----
            def g3_post():
                for bt in range(NB):
                    sca.activation(A[bt], A[bt], AF.Sigmoid,
                                   scale=b1[:, bt:bt + 1])      # r in place
                    dve.tensor_tensor(A[bt], A[bt], hx[bt], OP.mult)  # p

            gemm_fp8(w8p, w8_d["wz8"], hxT8, sc8_h, Bs,
                     inject={2: g3_post})
            for bt in range(NB):
                sq_accum_act(Bs[bt], n2ab_z[:, bt:bt + 1])
            # xT16 + G5's first slab (DMA early so G5 starts clean)
            nc.sync.dma_start(out=xT16, in_=xT16_d[:, :, :])
            nc.sync.dma_start(out=uw_s0, in_=uw16_d[0, :, :, :])

            # ---- G4: v4 = s_x*(x @ u_z.T) + bz -> D (DVE stt copies) ----
            gemm_fp8(w8p, w8_d["uz8"], xT8[0], sc8_x, Ds,
                     bias_bc=brz_bc, bias_off=D)
            for bt in range(NB):
                sq_accum_act(Ds[bt], n2ab_z[:, NB + bt:NB + bt + 1])
                sum_sq(Bs[bt], Ds[bt], n2s34[:, bt:bt + 1])
            dot_fin(d34, n2s34, n2ab_z[:, 0:NB], n2ab_z[:, NB:])

            # ---- z branch -> m2 -> z ----
            ua2, ub2, b2 = branch(n2ab_z, d34, "z")
            for bt in range(NB):
                combine(Bs[bt], Bs[bt], ua2[:, bt:bt + 1], Ds[bt],
                        ub2[:, bt:bt + 1])
                sca.activation(Ds[bt], Bs[bt], AF.Sigmoid,
                               scale=b2[:, bt:bt + 1])       # z -> D

            # ---- transpose p (A) -> pT on PE ----
            for bt in range(NB):
                for cp in range(KC // 4):
                    ps = ptr.tile([P, JB], F16, tag="tr", name="tr")
                    for k in range(4):
                        pe.transpose(
                            ps[:, k * P:(k + 1) * P],
                            A[bt][:, (cp * 4 + k) * P:(cp * 4 + k + 1) * P],
                            ident16,
                        )
                    dve.tensor_copy(
                        out=pT[:, cp * 4:cp * 4 + 4, bt * P:(bt + 1) * P],
                        in_=ps.rearrange("p (c b) -> p c b", c=4),
                    )
        slotAp.close()   # A free until phase C
        brzp.close()     # r/z biases dead after G4

        # =============== phase B: G5 + zh work + ww prefetch ===============
        wwpC = ctx.enter_context(ExitStack())
        wwpool = wwpC.enter_context(
            tc.tile_pool(name="wwp", bufs=1, side="right"))

        def prefetch_ww(js):
            def go():
                # gpsimd DMA queue: don't block G5's slab stream on sync
                wwslabs[js] = wwpool.tile([P, KC, JB], F16, tag=f"ww{js}",
                                          name=f"ww{js}")
                nc.gpsimd.dma_start(out=wwslabs[js],
                                    in_=ww16_d[js, :, :, :])
            return go

        def zh_work():
            # zh = z*hx -> B; |zh|^2 (ACT); <hx,zh> via sum (dhzh)
            for bt in range(NB):
                dve.tensor_tensor(Bs[bt], Ds[bt], hx[bt], OP.mult)
                sq_accum_act(Bs[bt], n2zh[:, bt:bt + 1])
                sum_sq(hx[bt], Bs[bt], n2szh[:, bt:bt + 1])
            dot_fin(dhzh, n2szh, n2xh[:, NB:], n2zh)

        # G5: v5 = s_x*(x @ u_w.T) + bw -> C (DVE stt copies)
        gemm_g5(wp16s, xT16, s_x, C, bw_bc, pre=uw_s0,
                inject={0: prefetch_ww(0), 2: zh_work, 3: prefetch_ww(1),
                        5: prefetch_ww(2), 6: prefetch_ww(3)})
        # |v5|^2 -> n2w pair layout cols (ACT)
        for bt in range(NB):
            vcol = (bt // 2) * 4 + 2 + (bt % 2)
            sq_accum_act(C[bt], n2w[:, vcol:vcol + 1])
        xp16.close()   # frees xT16 + G5 weight ring

        # =============== phase C: G6 bt-major + tail ===============
        with ExitStack() as phaseC:
            outp = phaseC.enter_context(
                tc.tile_pool(name="outp", bufs=1, side="right"))
            outt = [outp.tile([P, D], F32, tag=f"o{i}", name=f"o{i}")
                    for i in range(2)]
            A_big, A = slot_group(
                phaseC.enter_context(tc.tile_pool(name="slotA2", bufs=1)),
                "A2")

            b3s = [None, None]

            def pair_chain(g):
                """branch3 + m3 for bts (2g, 2g+1); m3 scale-halves on ACT."""
                sl = slice(4 * g, 4 * g + 4)
                ua3, ub3, b3 = branch(n2w[:, sl], d65[:, 2 * g:2 * g + 2],
                                      f"w{g}")
                for i, bt in enumerate((2 * g, 2 * g + 1)):
                    combine(A[bt], A[bt], ua3[:, i:i + 1], C[bt],
                            ub3[:, i:i + 1])    # m3 -> A
                b3s[g] = b3

            def pair_tanh(g):
                for i, bt in enumerate((2 * g, 2 * g + 1)):
                    sca.activation(C[bt], A[bt], AF.Tanh,
                                   scale=b3s[g][:, i:i + 1])     # q -> C

            def pair_postq(g):
                for bt in (2 * g, 2 * g + 1):
                    # dhq via sum identity: s = hx + q -> A (m3 dead)
                    dve.tensor_tensor(A[bt], hx[bt], C[bt], OP.add)
                    sca.activation(junk_act, A[bt], AF.Square,
                                   accum_out=n2shq[:, bt:bt + 1])
                    sq_accum_act(C[bt], n2q[:, bt:bt + 1])
                    # zq = z*q -> D (z dies)
                    dve.tensor_tensor(Ds[bt], Ds[bt], C[bt], OP.mult)
                    sq_accum_act(Ds[bt], n2zq[:, bt:bt + 1])
                    # dhzq = <hx,zq> = <zh,q>; dzhzq = <zh,zq> (DVE stt)
                    dot_accum(Bs[bt], C[bt], dhzq[:, bt:bt + 1])
                    dot_accum(Bs[bt], Ds[bt], dzhzq[:, bt:bt + 1])

            def tail_scalars(g):
                """d/e mobius scalar chain for pair g -> uo, c_zh, c_zq."""
                sl = slice(2 * g, 2 * g + 2)
                n2h = n2xh[:, NB + 2 * g:NB + 2 * g + 2]
                dq = dhq[:, sl]
                dve.tensor_tensor(dq, n2shq[:, sl], n2q[:, sl], OP.subtract)
                dve.tensor_tensor(dq, dq, n2h, OP.subtract)
                dve.tensor_scalar(dq, dq, 0.5, None, OP.mult)

                delta = t_poly(n2q[:, sl], w=2, name=f"delta{g}")
                thq2 = stile(2, name="thq2")
                dve.tensor_tensor(thq2, delta, delta, OP.mult)
                dve.tensor_tensor(thq2, thq2, n2q[:, sl], OP.mult)
                ua_d, ub_d, n2d = mobius(ones[:, sl], n2h, delta, thq2,
                                         dq, n2h, n2q[:, sl],
                                         neg_a=True, pfx=f"d{g}")
                beta_d = g_poly(n2d, w=2, name=f"beta_d{g}")

                n2e = stile(2, name="n2e")
                t1 = stile(2, name="te1")
                dve.tensor_tensor(t1, ua_d, ua_d, OP.mult)
                dve.tensor_tensor(t1, t1, n2zh[:, sl], OP.mult)
                t2 = stile(2, name="te2")
                dve.tensor_tensor(t2, ub_d, ub_d, OP.mult)
                dve.tensor_tensor(t2, t2, n2zq[:, sl], OP.mult)
                t3 = stile(2, name="te3")
                dve.tensor_tensor(t3, ua_d, ub_d, OP.mult)
                dve.tensor_tensor(t3, t3, dzhzq[:, sl], OP.mult)
                dve.scalar_tensor_tensor(n2e, t3, 2.0, t1, OP.mult, OP.add)
                dve.tensor_tensor(n2e, n2e, t2, OP.add)

                nt2 = stile(2, name="nt2")
                dve.tensor_tensor(nt2, beta_d, beta_d, OP.mult)
                dve.tensor_tensor(nt2, nt2, n2e, OP.mult)
                tt = t_poly(nt2, w=2, name=f"tt{g}")
                eps_s = stile(2, name="eps_s")
                dve.tensor_tensor(eps_s, tt, beta_d, OP.mult)
                tht2 = stile(2, name="tht2")
                dve.tensor_tensor(tht2, tt, tt, OP.mult)
                dve.tensor_tensor(tht2, tht2, nt2, OP.mult)
                dhe = stile(2, name="dhe")
                dve.tensor_tensor(dhe, ua_d, dhzh[:, sl], OP.mult)
                t4 = stile(2, name="te4")
                dve.tensor_tensor(t4, ub_d, dhzq[:, sl], OP.mult)
                dve.tensor_tensor(dhe, dhe, t4, OP.add)

                uo, vo, _ = mobius(ones[:, sl], n2h, eps_s, tht2, dhe,
                                   n2h, n2e, pfx=f"o{g}")
                dve.tensor_copy(out=uo_a[:, sl], in_=uo)
                dve.tensor_tensor(czh_a[:, sl], vo, ua_d, OP.mult)
                dve.tensor_tensor(czq_a[:, sl], vo, ub_d, OP.mult)

            def final_bt(bt):
                dve.tensor_scalar(Ds[bt], Ds[bt], czq_a[:, bt:bt + 1], None,
                                  OP.mult)
                dve.tensor_scalar(Bs[bt], Bs[bt], czh_a[:, bt:bt + 1], None,
                                  OP.mult)
                dve.tensor_tensor(Ds[bt], Ds[bt], Bs[bt], OP.add)
                o = outt[bt % 2]
                dve.scalar_tensor_tensor(o, hx[bt], uo_a[:, bt:bt + 1],
                                         Ds[bt], OP.mult, OP.add)
                nc.sync.dma_start(out=out_d[bt * P:(bt + 1) * P, :], in_=o)

            # ---- G6 bt-major: v6 = s_h*(p @ w.T) -> A (ACT copies) ----
            for bt in range(NB):
                for js in range(NJ):
                    ps = pmm.tile([P, JB], F32, tag="mm", name="mm")
                    for c in range(KC):
                        pe.matmul(
                            ps,
                            pT[:, c, bt * P:(bt + 1) * P],
                            wwslabs[js][:, c, :],
                            start=(c == 0),
                            stop=(c == KC - 1),
                        )
                    sca.activation(
                        out=A[bt][:, js * JB:(js + 1) * JB], in_=ps,
                        func=AF.Copy, scale=s_h[:, bt:bt + 1],
                    )
                col = (bt // 2) * 4 + (bt % 2)
                sq_accum_act(A[bt], n2w[:, col:col + 1])   # ACT: |v6|^2
                if bt == 3:
                    # pair0 epilogue: ready DVE work ahead of d65-bt3's wait
                    tail_scalars(0)
                    final_bt(0)
                    final_bt(1)
                dcol = (bt // 2) * 2 + (bt % 2)
                dot_accum(A[bt], C[bt], d65[:, dcol:dcol + 1])  # DVE stt
                if bt == 1:
                    pair_chain(0)
                if bt == 2:
                    pair_tanh(0)
                    pair_postq(0)
            pair_chain(1)
            pair_tanh(1)
            pair_postq(1)
            tail_scalars(1)
            final_bt(2)
            final_bt(3)

    nc.compile()
    return nc


_BUILD_LOCK = threading.Lock()
_NC_CACHE = {}


def _get_nc():
    with _BUILD_LOCK:
        if "nc" not in _NC_CACHE:
            _NC_CACHE["nc"] = _build()
        return _NC_CACHE["nc"]


def _prep_in_maps(inputs):
    f8 = ml_dtypes.float8_e4m3
    x = np.asarray(inputs["x"], dtype=np.float32)
    hx = np.asarray(inputs["hx"], dtype=np.float32)

    def swz_w(a, scale, dt, jb, nj):
        # W [D_out, D_in] -> wT [K=D_in, J=D_out] -> [nj, P, KC, jb] js-major
        wt = np.asarray(a, np.float32).T * scale
        if dt is f8:
            wt = np.clip(wt, -240, 240)
        return np.ascontiguousarray(
            wt.reshape(KC, P, nj, jb).transpose(2, 1, 0, 3)
        ).astype(dt)

    def swz_aT(aT, scale, dt):
        a = aT * scale
        if dt is f8:
            a = np.clip(a, -240, 240)
        return np.ascontiguousarray(
            a.reshape(KC, P, BL).transpose(1, 0, 2)
        ).astype(dt)

    def swz_nat(a):
        return np.ascontiguousarray(
            np.asarray(a, np.float32).reshape(NB, P, D).transpose(1, 0, 2)
        ).astype(np.float16)

    weights = {
        "wr8": swz_w(inputs["w_r"], SW, f8, JB, NJ),
        "ur8": swz_w(inputs["u_r_w"], SW, f8, JB, NJ),
        "wz8": swz_w(inputs["w_z"], SW, f8, JB, NJ),
        "uz8": swz_w(inputs["u_z_w"], SW, f8, JB, NJ),
        "uw16": swz_w(inputs["u_w"], 1.0, np.float16, JB5, NJ5),
        "ww16": swz_w(inputs["w"], 1.0, np.float16, JB, NJ),
    }
    brzw = np.concatenate([
        np.asarray(inputs["u_r_b"], np.float32),
        np.asarray(inputs["u_z_b"], np.float32),
        np.asarray(inputs["u_b"], np.float32),
    ]).reshape(1, 3 * D)
    biases = {"brzw16": brzw.astype(np.float16)}

    in_maps = []
    for c in range(N_CORES):
        xs = x[c * BL:(c + 1) * BL]
        hs = hx[c * BL:(c + 1) * BL]
        xsT = np.ascontiguousarray(xs.T)
        hsT = np.ascontiguousarray(hs.T)
        m = {
            "x16": swz_nat(xs),
            "hx16": swz_nat(hs),
            "xT8": swz_aT(xsT, SA, f8),
            "hxT8": swz_aT(hsT, SA, f8),
            "xT16": swz_aT(xsT, 1.0, np.float16),
        }
        m.update(weights)
        m.update(biases)
        in_maps.append(m)
    return in_maps


def kernel(**inputs: np.ndarray) -> np.ndarray:
    in_maps = _prep_in_maps(inputs)
    nc = _get_nc()
    res = run_bass_kernel_spmd(nc, in_maps, core_ids=list(range(N_CORES)))
    return np.concatenate([r["out"] for r in res.results], axis=0)
